# revision 10
# baseline (speedup 1.0000x reference)
"""Trainium2 Bass kernel for nn_MultiHeadAttention_67757404062370.

Sharding: data-parallel over batch (2) x tensor-parallel over heads (4 groups
of 4 heads) = 8 NeuronCores. Core c handles batch c//4, heads 4*(c%4)..4*(c%4)+3.

Device-side math (per core, everything in "transposed" layout, fp32r matmuls):
  qq^T = Wq_g^T q^T (+bq), kk^T likewise      [dcol, seq]
  vv   = v Wv_g                               [seq, dcol]  (bias folded on host)
  s^T[k,q]  = kh^T.T-contraction: lhsT=kh^T tile, rhs=qh^T  (K=64)
  u = exp(s/8); e = max(u,1)*keep             (exp(relu(x)) == max(exp(x),1))
  av^T[c,q] (+ sums row via ones column) = [vv|1]^T @ e
  att^T = e * (1/sums) ; out_av^T = av^T * (1/sums)
  outp[q,:] = sum_h out_av_h^T.T @ Wo_rows_h  (K=64 per head)
Host: gather, transpose att views, sum outp over the 4 head-group cores per
batch, add (bv @ Wo + bo), cast to float64.
"""

import sys

if "/opt/trn_rl_repo" not in sys.path:
    sys.path.insert(0, "/opt/trn_rl_repo")

import numpy as np
import ml_dtypes

import concourse.bacc as bacc
import concourse.tile as tile
from concourse import mybir
from concourse.bass_utils import run_bass_kernel_spmd

F = mybir.dt.float32
FR = mybir.dt.float32r
BF = mybir.dt.bfloat16
AF = mybir.ActivationFunctionType
OP = mybir.AluOpType

S = 1024      # seq len (post-slice) == v seq len
DIN = 1024    # d_model
NH = 4        # heads per core
DEP = 64      # head depth
DCOL = NH * DEP  # 256 projection columns per core
NT = S // 128    # 8 seq tiles
KT = DIN // 128  # 8 contraction tiles

_CACHE = {}
_DEBUG = False


def _build():
    nc = bacc.Bacc("TRN2", target_bir_lowering=False, debug=False, num_devices=8)

    d = {}
    d["qT"] = nc.dram_tensor("qT", [DIN, S], FR, kind="ExternalInput").ap()
    d["kT"] = nc.dram_tensor("kT", [DIN, S], FR, kind="ExternalInput").ap()
    d["vT"] = nc.dram_tensor("vT", [DIN, S], FR, kind="ExternalInput").ap()
    d["keepT"] = nc.dram_tensor("keepT", [S, S], BF, kind="ExternalInput").ap()
    d["wq"] = nc.dram_tensor("wq", [DIN, DCOL], FR, kind="ExternalInput").ap()
    d["wk"] = nc.dram_tensor("wk", [DIN, DCOL], FR, kind="ExternalInput").ap()
    d["wv"] = nc.dram_tensor("wv", [DIN, DCOL], FR, kind="ExternalInput").ap()
    d["wo"] = nc.dram_tensor("wo", [DCOL, DIN], FR, kind="ExternalInput").ap()
    d["bq"] = nc.dram_tensor("bq", [DEP, NH], F, kind="ExternalInput").ap()
    d["bk"] = nc.dram_tensor("bk", [DEP, NH], F, kind="ExternalInput").ap()
    d["attT"] = nc.dram_tensor("attT", [NH, S, S], F, kind="ExternalOutput").ap()
    d["outp"] = nc.dram_tensor("outp", [S, DIN], F, kind="ExternalOutput").ap()
    if _DEBUG:
        d["dbg_av"] = nc.dram_tensor("dbg_av", [NH, 65, S], F, kind="ExternalOutput").ap()
        d["dbg_oav"] = nc.dram_tensor("dbg_oav", [NH, DEP, S], F, kind="ExternalOutput").ap()
        d["dbg_vv"] = nc.dram_tensor("dbg_vv", [128, NT, NH * 65], F, kind="ExternalOutput").ap()

    with tile.TileContext(nc) as tc:
        _emit(nc, tc, d)
    nc.compile()
    return nc


def _emit(nc, tc, d):
    from contextlib import ExitStack

    ctx = ExitStack()
    with ctx:
        # ---------------- persistent tiles ----------------
        persist = ctx.enter_context(tc.tile_pool(name="persist", bufs=1))
        # per-head transposed projections [64, head, seq]
        qqT = persist.tile([DEP, NH, S], FR, tag="qqT")
        kkT = persist.tile([DEP, NH, S], FR, tag="kkT")
        # v projection w/ ones column: [kseq-in-tile, ktile, head*65]
        vv = persist.tile([128, NT, NH * 65], FR, tag="vv")
        # Wo rows per head [64, head, dmodel]
        wo_s = persist.tile([DEP, NH, DIN], FR, tag="wo")
        # mask keep^T [k-in-tile, ktile, q]
        keep_s = persist.tile([128, KT, S], BF, tag="keep")
        bq_s = persist.tile([DEP, NH], F, tag="bq")
        bk_s = persist.tile([DEP, NH], F, tag="bk")
        ones_sb = persist.tile([128, NH, 1], F, tag="ones")
        nc.vector.memset(ones_sb, 1.0)
        # normalized av^T per head, feeds Wo matmuls
        oav = [persist.tile([DEP, S], FR, tag=f"oav{h}", name=f"oav{h}") for h in range(NH)]

        nc.sync.dma_start(out=wo_s, in_=d["wo"].rearrange("(h p) n -> p h n", p=DEP))
        nc.sync.dma_start(out=keep_s, in_=d["keepT"].rearrange("(t p) q -> p t q", p=128))
        nc.sync.dma_start(out=bq_s, in_=d["bq"])
        nc.sync.dma_start(out=bk_s, in_=d["bk"])

        # ---------------- phase A: projections ----------------
        with tc.tile_pool(name="wqkv", bufs=1) as wpool, \
             tc.tile_pool(name="projin", bufs=4) as pin:
            wq_s = wpool.tile([128, KT, DCOL], FR, tag="wq")
            wk_s = wpool.tile([128, KT, DCOL], FR, tag="wk")
            wv_s = wpool.tile([128, KT, DCOL], FR, tag="wv")
            nc.sync.dma_start(out=wq_s, in_=d["wq"].rearrange("(t p) c -> p t c", p=128))
            nc.sync.dma_start(out=wk_s, in_=d["wk"].rearrange("(t p) c -> p t c", p=128))
            nc.sync.dma_start(out=wv_s, in_=d["wv"].rearrange("(t p) c -> p t c", p=128))

            # A1: qq^T, kk^T   (psum [128,1024] x2 each = 8 banks)
            with tc.tile_pool(name="psA1", bufs=4, space="PSUM") as psA1:
                ps_q = [psA1.tile([128, S], F, tag="psA1", name=f"psq{i}") for i in range(2)]
                ps_k = [psA1.tile([128, S], F, tag="psA1", name=f"psk{i}") for i in range(2)]
                for kt in range(KT):
                    qin = pin.tile([128, S], FR, tag="pin")
                    nc.sync.dma_start(out=qin, in_=d["qT"][kt * 128:(kt + 1) * 128, :])
                    kin = pin.tile([128, S], FR, tag="pin")
                    nc.sync.dma_start(out=kin, in_=d["kT"][kt * 128:(kt + 1) * 128, :])
                    for dc in range(2):
                        for cc in range(2):
                            sl = slice(cc * 512, (cc + 1) * 512)
                            nc.tensor.matmul(
                                ps_q[dc][:, sl],
                                wq_s[:, kt, dc * 128:(dc + 1) * 128],
                                qin[:, sl],
                                start=(kt == 0), stop=(kt == KT - 1))
                            nc.tensor.matmul(
                                ps_k[dc][:, sl],
                                wk_s[:, kt, dc * 128:(dc + 1) * 128],
                                kin[:, sl],
                                start=(kt == 0), stop=(kt == KT - 1))
                # evacuate with bias add (Identity allows per-partition bias AP)
                for dc in range(2):
                    for hf in range(2):
                        h = dc * 2 + hf
                        rows = slice(hf * DEP, (hf + 1) * DEP)
                        nc.scalar.activation(
                            out=qqT[:, h, :], in_=ps_q[dc][rows, :],
                            func=AF.Identity, bias=bq_s[:, h:h + 1], scale=1.0)
                        nc.scalar.activation(
                            out=kkT[:, h, :], in_=ps_k[dc][rows, :],
                            func=AF.Identity, bias=bk_s[:, h:h + 1], scale=1.0)

            # A2: vv (normal layout) + ones columns
            # NB: matmul start=True clears the whole PSUM bank, so every
            # accumulation group needs its own bank (8 tiles of [128, 256]).
            with tc.tile_pool(name="psA2", bufs=1, space="PSUM") as psA2:
                ps_v = [psA2.tile([128, 256], F, tag=f"psA2{i}", name=f"psv{i}") for i in range(NT)]
                for kt in range(KT):
                    vin = pin.tile([128, S], FR, tag="pin")
                    nc.sync.dma_start(out=vin, in_=d["vT"][kt * 128:(kt + 1) * 128, :])
                    for st in range(NT):
                        nc.tensor.matmul(
                            ps_v[st],
                            vin[:, st * 128:(st + 1) * 128],
                            wv_s[:, kt, :],
                            start=(kt == 0), stop=(kt == KT - 1))
                for st in range(NT):
                    src = ps_v[st]
                    dst = vv[:, st, :].rearrange("p (h x) -> p h x", h=NH)
                    nc.scalar.activation(
                        out=dst[:, :, 0:DEP],
                        in_=src.rearrange("p (h x) -> p h x", h=NH),
                        func=AF.Copy, scale=1.0)
                    nc.vector.tensor_copy(out=dst[:, :, DEP:DEP + 1], in_=ones_sb)

        # ---------------- phase B: heads ----------------
        with tc.tile_pool(name="escore", bufs=9) as epool, \
             tc.tile_pool(name="utile", bufs=3) as upool, \
             tc.tile_pool(name="attsb", bufs=4) as apool, \
             tc.tile_pool(name="rtiles", bufs=2) as rpool, \
             tc.tile_pool(name="psS", bufs=2, space="PSUM") as psS, \
             tc.tile_pool(name="psAV", bufs=2, space="PSUM") as psAV:
            for h in range(NH):
                e_tiles = []
                av = psAV.tile([65, S], F, tag="av")
                for kt in range(KT):
                    ps = psS.tile([128, S], F, tag="score")
                    for cc in range(2):
                        sl = slice(cc * 512, (cc + 1) * 512)
                        nc.tensor.matmul(
                            ps[:, sl],
                            kkT[:, h, kt * 128:(kt + 1) * 128],
                            qqT[:, h, sl],
                            start=True, stop=True)
                    u = upool.tile([128, S], F, tag="u")
                    nc.scalar.activation(out=u, in_=ps, func=AF.Exp, scale=0.125)
                    e = epool.tile([128, S], FR, tag="e")
                    nc.vector.scalar_tensor_tensor(
                        out=e, in0=u, scalar=1.0, in1=keep_s[:, kt, :],
                        op0=OP.max, op1=OP.mult)
                    e_tiles.append(e)
                    for cc in range(2):
                        sl = slice(cc * 512, (cc + 1) * 512)
                        nc.tensor.matmul(
                            av[:, sl],
                            vv[:, kt, h * 65:(h + 1) * 65],
                            e[:, sl],
                            start=(kt == 0), stop=(kt == KT - 1))
                # reciprocal of sums, broadcast to all partitions
                srow = rpool.tile([1, S], F, tag="srow")
                nc.scalar.activation(out=srow, in_=av[64:65, :], func=AF.Copy, scale=1.0)
                sbc = rpool.tile([128, S], F, tag="sbc")
                nc.gpsimd.partition_broadcast(sbc, srow)
                rb = rpool.tile([128, S], F, tag="rb")
                nc.vector.reciprocal_approx_fast(out=rb, in_=sbc)
                # normalized attention out (att^T layout), split DVE/GPSIMD
                for kt in range(KT):
                    att_sb = apool.tile([128, S], F, tag="att")
                    eng = nc.vector if kt % 2 == 0 else nc.gpsimd
                    eng.tensor_tensor(
                        out=att_sb, in0=e_tiles[kt].bitcast(F), in1=rb, op=OP.mult)
                    nc.sync.dma_start(out=d["attT"][h, kt * 128:(kt + 1) * 128, :], in_=att_sb)
                # normalized av^T
                nc.vector.tensor_tensor(
                    out=oav[h], in0=av[0:DEP, :], in1=rb[0:DEP, :], op=OP.mult)
                if _DEBUG:
                    av_dump = apool.tile([65, S], F, tag="avdump", name=f"avd{h}")
                    nc.scalar.activation(out=av_dump, in_=av, func=AF.Copy, scale=1.0)
                    nc.sync.dma_start(out=d["dbg_av"][h], in_=av_dump)
                    nc.sync.dma_start(out=d["dbg_oav"][h], in_=oav[h].bitcast(F))

        if _DEBUG:
            nc.sync.dma_start(out=d["dbg_vv"], in_=vv.bitcast(F))

        # ---------------- phase C: output projection ----------------
        with tc.tile_pool(name="osb", bufs=3) as opool, \
             tc.tile_pool(name="psO", bufs=4, space="PSUM") as psO:
            for qt in range(NT):
                out_sb = opool.tile([128, DIN], F, tag="osb")
                for nch in range(2):
                    po = psO.tile([128, 512], F, tag="po")
                    for h in range(NH):
                        nc.tensor.matmul(
                            po,
                            oav[h][:, qt * 128:(qt + 1) * 128],
                            wo_s[:, h, nch * 512:(nch + 1) * 512],
                            start=(h == 0), stop=(h == NH - 1))
                    nc.scalar.activation(
                        out=out_sb[:, nch * 512:(nch + 1) * 512], in_=po,
                        func=AF.Copy, scale=1.0)
                nc.sync.dma_start(out=d["outp"][qt * 128:(qt + 1) * 128, :], in_=out_sb)


def _get_nc():
    if "nc" not in _CACHE:
        _CACHE["nc"] = _build()
    return _CACHE["nc"]


def kernel(v, k, q, mask, Wq0, bq0, Wk0, bk0, Wv, bv, Wo, bo):
    v = np.asarray(v, dtype=np.float32)
    k = np.asarray(k, dtype=np.float32)
    q = np.asarray(q, dtype=np.float32)
    mask = np.asarray(mask)
    Wq0 = np.asarray(Wq0, dtype=np.float32)
    Wk0 = np.asarray(Wk0, dtype=np.float32)
    Wv = np.asarray(Wv, dtype=np.float32)
    Wo = np.asarray(Wo, dtype=np.float32)
    bq0 = np.asarray(bq0, dtype=np.float32)
    bk0 = np.asarray(bk0, dtype=np.float32)
    bv = np.asarray(bv, dtype=np.float32)
    bo = np.asarray(bo, dtype=np.float32)
    B = v.shape[0]
    HTOT = 16

    nc = _get_nc()

    # host-side shard prep
    per_batch = []
    for b in range(B):
        per_batch.append({
            "qT": np.ascontiguousarray(q[b, 1:, :].T),
            "kT": np.ascontiguousarray(k[b, :-1, :].T),
            "vT": np.ascontiguousarray(v[b].T),
            "keepT": np.ascontiguousarray(
                (1 - mask[b]).T.astype(np.float32)).astype(ml_dtypes.bfloat16),
        })
    in_maps = []
    for c in range(8):
        b, g = c // 4, c % 4
        cols = slice(g * DCOL, (g + 1) * DCOL)
        m = dict(per_batch[b])
        m["wq"] = np.ascontiguousarray(Wq0[:, cols])
        m["wk"] = np.ascontiguousarray(Wk0[:, cols])
        m["wv"] = np.ascontiguousarray(Wv[:, cols])
        m["wo"] = np.ascontiguousarray(Wo[cols, :])
        m["bq"] = np.ascontiguousarray(bq0[cols].reshape(NH, DEP).T)
        m["bk"] = np.ascontiguousarray(bk0[cols].reshape(NH, DEP).T)
        in_maps.append(m)

    res = run_bass_kernel_spmd(nc, in_maps, core_ids=list(range(8)))

    att = np.empty((B, HTOT, S, S), dtype=np.float64)
    out = np.empty((B, S, DIN), dtype=np.float64)
    bias_row = (bv.astype(np.float64) @ Wo.astype(np.float64)) + bo.astype(np.float64)
    for b in range(B):
        acc = None
        for g in range(4):
            r = res.results[b * 4 + g]
            attT = r["attT"]
            for hl in range(NH):
                att[b, g * NH + hl] = attT[hl].T
            acc = r["outp"].astype(np.float64) if acc is None else acc + r["outp"]
        out[b] = acc + bias_row[None, :]
    return out, att


# revision 14
# speedup vs baseline: 1.0283x; 1.0283x over previous
"""Trainium2 Bass kernel for nn_MultiHeadAttention_67757404062370.

Sharding: data-parallel over batch (2) x tensor-parallel over heads (4 groups
of 4 heads) = 8 NeuronCores. Core c handles batch c//4, heads 4*(c%4)..4*(c%4)+3.

Device-side math (per core, everything in "transposed" layout, fp32r matmuls):
  qq^T = Wq_g^T q^T (+bq), kk^T likewise      [dcol, seq]
  vv   = v Wv_g                               [seq, dcol]  (bias folded on host)
  s^T[k,q]  = kh^T.T-contraction: lhsT=kh^T tile, rhs=qh^T  (K=64)
  u = exp(s/8); e = max(u,1)*keep             (exp(relu(x)) == max(exp(x),1))
  av^T[c,q] (+ sums row via ones column) = [vv|1]^T @ e
  att^T = e * (1/sums) ; out_av^T = av^T * (1/sums)
  outp[q,:] = sum_h out_av_h^T.T @ Wo_rows_h  (K=64 per head)
Host: gather, transpose att views, sum outp over the 4 head-group cores per
batch, add (bv @ Wo + bo), cast to float64.
"""

import sys

if "/opt/trn_rl_repo" not in sys.path:
    sys.path.insert(0, "/opt/trn_rl_repo")

import numpy as np
import ml_dtypes

import concourse.bacc as bacc
import concourse.tile as tile
from concourse import mybir
from concourse.bass_utils import run_bass_kernel_spmd

F = mybir.dt.float32
FR = mybir.dt.float32r
BF = mybir.dt.bfloat16
AF = mybir.ActivationFunctionType
OP = mybir.AluOpType

S = 1024      # seq len (post-slice) == v seq len
DIN = 1024    # d_model
NH = 4        # heads per core
DEP = 64      # head depth
DCOL = NH * DEP  # 256 projection columns per core
NT = S // 128    # 8 seq tiles
KT = DIN // 128  # 8 contraction tiles

_CACHE = {}
_DEBUG = False


def _build():
    nc = bacc.Bacc("TRN2", target_bir_lowering=False, debug=False, num_devices=8)

    d = {}
    d["qT"] = nc.dram_tensor("qT", [DIN, S], FR, kind="ExternalInput").ap()
    d["kT"] = nc.dram_tensor("kT", [DIN, S], FR, kind="ExternalInput").ap()
    d["vT"] = nc.dram_tensor("vT", [DIN, S], FR, kind="ExternalInput").ap()
    d["keepT"] = nc.dram_tensor("keepT", [S, S], BF, kind="ExternalInput").ap()
    d["wq"] = nc.dram_tensor("wq", [DIN, DCOL], FR, kind="ExternalInput").ap()
    d["wk"] = nc.dram_tensor("wk", [DIN, DCOL], FR, kind="ExternalInput").ap()
    d["wv"] = nc.dram_tensor("wv", [DIN, DCOL], FR, kind="ExternalInput").ap()
    d["wo"] = nc.dram_tensor("wo", [DCOL, DIN], FR, kind="ExternalInput").ap()
    d["bq"] = nc.dram_tensor("bq", [DEP, NH], F, kind="ExternalInput").ap()
    d["bk"] = nc.dram_tensor("bk", [DEP, NH], F, kind="ExternalInput").ap()
    d["attT"] = nc.dram_tensor("attT", [NH, S, S], F, kind="ExternalOutput").ap()
    d["outp"] = nc.dram_tensor("outp", [S, DIN], F, kind="ExternalOutput").ap()
    if _DEBUG:
        d["dbg_av"] = nc.dram_tensor("dbg_av", [NH, 65, S], F, kind="ExternalOutput").ap()
        d["dbg_oav"] = nc.dram_tensor("dbg_oav", [NH, DEP, S], F, kind="ExternalOutput").ap()
        d["dbg_vv"] = nc.dram_tensor("dbg_vv", [128, NT, NH * 65], F, kind="ExternalOutput").ap()

    with tile.TileContext(nc) as tc:
        _emit(nc, tc, d)
    nc.compile()
    return nc


def _emit(nc, tc, d):
    from contextlib import ExitStack

    ctx = ExitStack()
    with ctx:
        # ---------------- persistent tiles ----------------
        persist = ctx.enter_context(tc.tile_pool(name="persist", bufs=1))
        # per-head transposed projections [64, head, seq]
        qqT = persist.tile([DEP, NH, S], FR, tag="qqT")
        kkT = persist.tile([DEP, NH, S], FR, tag="kkT")
        # v projection w/ ones column: [kseq-in-tile, ktile, head*65]
        vv = persist.tile([128, NT, NH * 65], FR, tag="vv")
        # Wo rows per head [64, head, dmodel]
        wo_s = persist.tile([DEP, NH, DIN], FR, tag="wo")
        # mask keep^T [k-in-tile, ktile, q]
        keep_s = persist.tile([128, KT, S], BF, tag="keep")
        bq_s = persist.tile([DEP, NH], F, tag="bq")
        bk_s = persist.tile([DEP, NH], F, tag="bk")
        ones_sb = persist.tile([128, NH, 1], F, tag="ones")
        nc.vector.memset(ones_sb, 1.0)
        # normalized av^T per head, feeds Wo matmuls
        oav = [persist.tile([DEP, S], FR, tag=f"oav{h}", name=f"oav{h}") for h in range(NH)]

        nc.sync.dma_start(out=bq_s, in_=d["bq"])
        nc.sync.dma_start(out=bk_s, in_=d["bk"])

        # ---------------- phase A: projections ----------------
        with tc.tile_pool(name="wqkv", bufs=1) as wpool, \
             tc.tile_pool(name="projin", bufs=4) as pin:
            wq_s = wpool.tile([128, KT, DCOL], FR, tag="wq")
            wk_s = wpool.tile([128, KT, DCOL], FR, tag="wk")
            wv_s = wpool.tile([128, KT, DCOL], FR, tag="wv")

            # A1: qq^T, kk^T   (psum [128,1024] x2 each = 8 banks)
            # Weights and inputs stream per k-tile so the first matmuls can
            # start as soon as one slice of each has landed.
            with tc.tile_pool(name="psA1", bufs=4, space="PSUM") as psA1:
                ps_q = [psA1.tile([128, S], F, tag="psA1", name=f"psq{i}") for i in range(2)]
                ps_k = [psA1.tile([128, S], F, tag="psA1", name=f"psk{i}") for i in range(2)]
                for kt in range(KT):
                    rows = slice(kt * 128, (kt + 1) * 128)
                    nc.sync.dma_start(out=wq_s[:, kt, :], in_=d["wq"][rows, :])
                    nc.sync.dma_start(out=wk_s[:, kt, :], in_=d["wk"][rows, :])
                    qin = pin.tile([128, S], FR, tag="pin")
                    nc.sync.dma_start(out=qin, in_=d["qT"][rows, :])
                    kin = pin.tile([128, S], FR, tag="pin")
                    nc.sync.dma_start(out=kin, in_=d["kT"][rows, :])
                    for dc in range(2):
                        for cc in range(2):
                            sl = slice(cc * 512, (cc + 1) * 512)
                            nc.tensor.matmul(
                                ps_q[dc][:, sl],
                                wq_s[:, kt, dc * 128:(dc + 1) * 128],
                                qin[:, sl],
                                start=(kt == 0), stop=(kt == KT - 1))
                            nc.tensor.matmul(
                                ps_k[dc][:, sl],
                                wk_s[:, kt, dc * 128:(dc + 1) * 128],
                                kin[:, sl],
                                start=(kt == 0), stop=(kt == KT - 1))
                # evacuate with bias add (Identity allows per-partition bias AP)
                for dc in range(2):
                    for hf in range(2):
                        h = dc * 2 + hf
                        rows = slice(hf * DEP, (hf + 1) * DEP)
                        nc.scalar.activation(
                            out=qqT[:, h, :], in_=ps_q[dc][rows, :],
                            func=AF.Identity, bias=bq_s[:, h:h + 1], scale=1.0)
                        nc.scalar.activation(
                            out=kkT[:, h, :], in_=ps_k[dc][rows, :],
                            func=AF.Identity, bias=bk_s[:, h:h + 1], scale=1.0)

            # A2: vv (normal layout) + ones columns
            # NB: matmul start=True clears the whole PSUM bank, so every
            # accumulation group needs its own bank (8 tiles of [128, 256]).
            with tc.tile_pool(name="psA2", bufs=1, space="PSUM") as psA2:
                ps_v = [psA2.tile([128, 256], F, tag=f"psA2{i}", name=f"psv{i}") for i in range(NT)]
                for kt in range(KT):
                    rows = slice(kt * 128, (kt + 1) * 128)
                    nc.sync.dma_start(out=wv_s[:, kt, :], in_=d["wv"][rows, :])
                    vin = pin.tile([128, S], FR, tag="pin")
                    nc.sync.dma_start(out=vin, in_=d["vT"][rows, :])
                    # interleave the mask + wo loads behind the v stream
                    nc.sync.dma_start(
                        out=keep_s[:, kt, :],
                        in_=d["keepT"][kt * 128:(kt + 1) * 128, :])
                    if kt < NH:
                        nc.sync.dma_start(
                            out=wo_s[:, kt, :],
                            in_=d["wo"][kt * DEP:(kt + 1) * DEP, :])
                    for st in range(NT):
                        nc.tensor.matmul(
                            ps_v[st],
                            vin[:, st * 128:(st + 1) * 128],
                            wv_s[:, kt, :],
                            start=(kt == 0), stop=(kt == KT - 1))
                for st in range(NT):
                    src = ps_v[st]
                    dst = vv[:, st, :].rearrange("p (h x) -> p h x", h=NH)
                    nc.scalar.activation(
                        out=dst[:, :, 0:DEP],
                        in_=src.rearrange("p (h x) -> p h x", h=NH),
                        func=AF.Copy, scale=1.0)
                    nc.vector.tensor_copy(out=dst[:, :, DEP:DEP + 1], in_=ones_sb)

        # ---------------- phase B: heads ----------------
        with tc.tile_pool(name="escore", bufs=10) as epool, \
             tc.tile_pool(name="utile", bufs=3) as upool, \
             tc.tile_pool(name="attsb", bufs=4) as apool, \
             tc.tile_pool(name="rtiles", bufs=2) as rpool, \
             tc.tile_pool(name="psS", bufs=2, space="PSUM") as psS, \
             tc.tile_pool(name="psAV", bufs=2, space="PSUM") as psAV:
            for h in range(NH):
                e_tiles = []
                av = psAV.tile([65, S], F, tag="av")
                for kt in range(KT):
                    ps = psS.tile([128, S], F, tag="score")
                    for cc in range(2):
                        sl = slice(cc * 512, (cc + 1) * 512)
                        nc.tensor.matmul(
                            ps[:, sl],
                            kkT[:, h, kt * 128:(kt + 1) * 128],
                            qqT[:, h, sl],
                            start=True, stop=True)
                    u = upool.tile([128, S], F, tag="u")
                    nc.scalar.activation(out=u, in_=ps, func=AF.Exp, scale=0.125)
                    e = epool.tile([128, S], FR, tag="e")
                    nc.vector.scalar_tensor_tensor(
                        out=e, in0=u, scalar=1.0, in1=keep_s[:, kt, :],
                        op0=OP.max, op1=OP.mult)
                    e_tiles.append(e)
                    for cc in range(2):
                        sl = slice(cc * 512, (cc + 1) * 512)
                        nc.tensor.matmul(
                            av[:, sl],
                            vv[:, kt, h * 65:(h + 1) * 65],
                            e[:, sl],
                            start=(kt == 0), stop=(kt == KT - 1))
                # reciprocal of sums, broadcast to all partitions
                srow = rpool.tile([1, S], F, tag="srow")
                nc.scalar.activation(out=srow, in_=av[64:65, :], func=AF.Copy, scale=1.0)
                sbc = rpool.tile([128, S], F, tag="sbc")
                nc.gpsimd.partition_broadcast(sbc, srow)
                rb = rpool.tile([128, S], F, tag="rb")
                nc.vector.reciprocal_approx_fast(out=rb, in_=sbc)
                # normalized attention out (att^T layout), split DVE/GPSIMD
                for kt in range(KT):
                    att_sb = apool.tile([128, S], F, tag="att")
                    eng = nc.vector if kt % 2 == 0 else nc.gpsimd
                    eng.tensor_tensor(
                        out=att_sb, in0=e_tiles[kt].bitcast(F), in1=rb, op=OP.mult)
                    nc.sync.dma_start(out=d["attT"][h, kt * 128:(kt + 1) * 128, :], in_=att_sb)
                # normalized av^T
                nc.vector.tensor_tensor(
                    out=oav[h], in0=av[0:DEP, :], in1=rb[0:DEP, :], op=OP.mult)
                if _DEBUG:
                    av_dump = apool.tile([65, S], F, tag="avdump", name=f"avd{h}")
                    nc.scalar.activation(out=av_dump, in_=av, func=AF.Copy, scale=1.0)
                    nc.sync.dma_start(out=d["dbg_av"][h], in_=av_dump)
                    nc.sync.dma_start(out=d["dbg_oav"][h], in_=oav[h].bitcast(F))

        if _DEBUG:
            nc.sync.dma_start(out=d["dbg_vv"], in_=vv.bitcast(F))

        # ---------------- phase C: output projection ----------------
        with tc.tile_pool(name="osb", bufs=3) as opool, \
             tc.tile_pool(name="psO", bufs=4, space="PSUM") as psO:
            for qt in range(NT):
                out_sb = opool.tile([128, DIN], F, tag="osb")
                for nch in range(2):
                    po = psO.tile([128, 512], F, tag="po")
                    for h in range(NH):
                        nc.tensor.matmul(
                            po,
                            oav[h][:, qt * 128:(qt + 1) * 128],
                            wo_s[:, h, nch * 512:(nch + 1) * 512],
                            start=(h == 0), stop=(h == NH - 1))
                    nc.scalar.activation(
                        out=out_sb[:, nch * 512:(nch + 1) * 512], in_=po,
                        func=AF.Copy, scale=1.0)
                nc.sync.dma_start(out=d["outp"][qt * 128:(qt + 1) * 128, :], in_=out_sb)


def _get_nc():
    if "nc" not in _CACHE:
        _CACHE["nc"] = _build()
    return _CACHE["nc"]


def kernel(v, k, q, mask, Wq0, bq0, Wk0, bk0, Wv, bv, Wo, bo):
    v = np.asarray(v, dtype=np.float32)
    k = np.asarray(k, dtype=np.float32)
    q = np.asarray(q, dtype=np.float32)
    mask = np.asarray(mask)
    Wq0 = np.asarray(Wq0, dtype=np.float32)
    Wk0 = np.asarray(Wk0, dtype=np.float32)
    Wv = np.asarray(Wv, dtype=np.float32)
    Wo = np.asarray(Wo, dtype=np.float32)
    bq0 = np.asarray(bq0, dtype=np.float32)
    bk0 = np.asarray(bk0, dtype=np.float32)
    bv = np.asarray(bv, dtype=np.float32)
    bo = np.asarray(bo, dtype=np.float32)
    B = v.shape[0]
    HTOT = 16

    nc = _get_nc()

    # host-side shard prep
    per_batch = []
    for b in range(B):
        per_batch.append({
            "qT": np.ascontiguousarray(q[b, 1:, :].T),
            "kT": np.ascontiguousarray(k[b, :-1, :].T),
            "vT": np.ascontiguousarray(v[b].T),
            "keepT": np.ascontiguousarray(
                (1 - mask[b]).T.astype(np.float32)).astype(ml_dtypes.bfloat16),
        })
    in_maps = []
    for c in range(8):
        b, g = c // 4, c % 4
        cols = slice(g * DCOL, (g + 1) * DCOL)
        m = dict(per_batch[b])
        m["wq"] = np.ascontiguousarray(Wq0[:, cols])
        m["wk"] = np.ascontiguousarray(Wk0[:, cols])
        m["wv"] = np.ascontiguousarray(Wv[:, cols])
        m["wo"] = np.ascontiguousarray(Wo[cols, :])
        m["bq"] = np.ascontiguousarray(bq0[cols].reshape(NH, DEP).T)
        m["bk"] = np.ascontiguousarray(bk0[cols].reshape(NH, DEP).T)
        in_maps.append(m)

    res = run_bass_kernel_spmd(nc, in_maps, core_ids=list(range(8)))

    att = np.empty((B, HTOT, S, S), dtype=np.float64)
    out = np.empty((B, S, DIN), dtype=np.float64)
    bias_row = (bv.astype(np.float64) @ Wo.astype(np.float64)) + bo.astype(np.float64)
    for b in range(B):
        acc = None
        for g in range(4):
            r = res.results[b * 4 + g]
            attT = r["attT"]
            for hl in range(NH):
                att[b, g * NH + hl] = attT[hl].T
            acc = r["outp"].astype(np.float64) if acc is None else acc + r["outp"]
        out[b] = acc + bias_row[None, :]
    return out, att


# revision 15
# speedup vs baseline: 1.1267x; 1.0957x over previous
"""Trainium2 Bass kernel for nn_MultiHeadAttention_67757404062370.

Sharding: data-parallel over batch (2) x tensor-parallel over heads (4 groups
of 4 heads) = 8 NeuronCores. Core c handles batch c//4, heads 4*(c%4)..4*(c%4)+3.

Device-side math (per core, everything in "transposed" layout):
  vv   = v Wv_g  (fp32r matmul)               [seq, dcol] + ones cols, fp16
  qq^T = Wq_g^T q^T (+bq), kk^T likewise      [dcol, seq], fp32r
  s^T[k,q]  = lhsT=kh^T tile, rhs=qh^T        (K=64, fp32r)
  u = exp(s/8) fp16; e = max(u,1)*keep fp16   (exp(relu(x)) == max(exp(x),1))
  av^T[c,q] (+ sums row via ones column) = [vv|1]^T @ e   (fp16 matmul)
  att^T = e * (1/sums) fp16 ; oav^T = av^T * (1/sums) fp32r
  outp[q,:] = sum_h oav_h^T.T @ Wo_rows_h  (K=64 per head, fp32r)
Host: gather, transpose att views, sum outp over the 4 head-group cores per
batch, add (bv @ Wo + bo), cast to float64.
"""

import sys

if "/opt/trn_rl_repo" not in sys.path:
    sys.path.insert(0, "/opt/trn_rl_repo")

import numpy as np
import ml_dtypes

import concourse.bacc as bacc
import concourse.tile as tile
from concourse import mybir
from concourse.bass_utils import run_bass_kernel_spmd

F = mybir.dt.float32
FR = mybir.dt.float32r
F16 = mybir.dt.float16
AF = mybir.ActivationFunctionType
OP = mybir.AluOpType

S = 1024      # seq len (post-slice) == v seq len
DIN = 1024    # d_model
NH = 4        # heads per core
DEP = 64      # head depth
DCOL = NH * DEP  # 256 projection columns per core
NT = S // 128    # 8 seq tiles
KT = DIN // 128  # 8 contraction tiles

_CACHE = {}
_DEBUG = False


def _build():
    nc = bacc.Bacc("TRN2", target_bir_lowering=False, debug=False, num_devices=8)

    d = {}
    d["qT"] = nc.dram_tensor("qT", [DIN, S], FR, kind="ExternalInput").ap()
    d["kT"] = nc.dram_tensor("kT", [DIN, S], FR, kind="ExternalInput").ap()
    d["vT"] = nc.dram_tensor("vT", [DIN, S], FR, kind="ExternalInput").ap()
    d["keepT"] = nc.dram_tensor("keepT", [S, S], F16, kind="ExternalInput").ap()
    d["wq"] = nc.dram_tensor("wq", [DIN, DCOL], FR, kind="ExternalInput").ap()
    d["wk"] = nc.dram_tensor("wk", [DIN, DCOL], FR, kind="ExternalInput").ap()
    d["wv"] = nc.dram_tensor("wv", [DIN, DCOL], FR, kind="ExternalInput").ap()
    d["wo"] = nc.dram_tensor("wo", [DCOL, DIN], FR, kind="ExternalInput").ap()
    d["bq"] = nc.dram_tensor("bq", [DEP, NH], F, kind="ExternalInput").ap()
    d["bk"] = nc.dram_tensor("bk", [DEP, NH], F, kind="ExternalInput").ap()
    d["attT"] = nc.dram_tensor("attT", [NH, S, S], F16, kind="ExternalOutput").ap()
    d["outp"] = nc.dram_tensor("outp", [S, DIN], F, kind="ExternalOutput").ap()
    if _DEBUG:
        d["dbg_av"] = nc.dram_tensor("dbg_av", [NH, 65, S], F, kind="ExternalOutput").ap()
        d["dbg_oav"] = nc.dram_tensor("dbg_oav", [NH, DEP, S], F, kind="ExternalOutput").ap()

    with tile.TileContext(nc) as tc:
        _emit(nc, tc, d)
    nc.compile()
    return nc


def _emit(nc, tc, d):
    from contextlib import ExitStack

    ctx = ExitStack()
    with ctx:
        # ---------------- persistent tiles ----------------
        persist = ctx.enter_context(tc.tile_pool(name="persist", bufs=1))
        qqT = persist.tile([DEP, NH, S], FR, tag="qqT")
        kkT = persist.tile([DEP, NH, S], FR, tag="kkT")
        # v projection w/ ones column: [kseq-in-tile, ktile, head*65], fp16
        vv = persist.tile([128, NT, NH * 65], F16, tag="vv")
        wo_s = persist.tile([DEP, NH, DIN], FR, tag="wo")
        keep_s = persist.tile([128, KT, S], F16, tag="keep")
        bq_s = persist.tile([DEP, NH], F, tag="bq")
        bk_s = persist.tile([DEP, NH], F, tag="bk")
        ones_sb = persist.tile([128, NH, 1], F, tag="ones")
        nc.vector.memset(ones_sb, 1.0)
        oav = [persist.tile([DEP, S], FR, tag=f"oav{h}", name=f"oav{h}") for h in range(NH)]

        nc.sync.dma_start(out=bq_s, in_=d["bq"])
        nc.sync.dma_start(out=bk_s, in_=d["bk"])

        # ---------------- phase A: projections ----------------
        # v first: its PSUM banks (8) free up before q/k accumulation needs
        # them, and vv is ready early for the first head's AV matmuls.
        with tc.tile_pool(name="wqkv", bufs=1) as wpool, \
             tc.tile_pool(name="projin", bufs=4) as pin:
            wq_s = wpool.tile([128, KT, DCOL], FR, tag="wq")
            wk_s = wpool.tile([128, KT, DCOL], FR, tag="wk")
            wv_s = wpool.tile([128, KT, DCOL], FR, tag="wv")

            # A2: vv (normal layout) + ones columns
            # NB: matmul start=True clears the whole PSUM bank, so every
            # accumulation group needs its own bank (8 tiles of [128, 256]).
            with tc.tile_pool(name="psA2", bufs=1, space="PSUM") as psA2:
                ps_v = [psA2.tile([128, 256], F, tag=f"psA2{i}", name=f"psv{i}") for i in range(NT)]
                for kt in range(KT):
                    rows = slice(kt * 128, (kt + 1) * 128)
                    nc.sync.dma_start(out=wv_s[:, kt, :], in_=d["wv"][rows, :])
                    vin = pin.tile([128, S], FR, tag="pin")
                    nc.sync.dma_start(out=vin, in_=d["vT"][rows, :])
                    for st in range(NT):
                        nc.tensor.matmul(
                            ps_v[st],
                            vin[:, st * 128:(st + 1) * 128],
                            wv_s[:, kt, :],
                            start=(kt == 0), stop=(kt == KT - 1))
                for st in range(NT):
                    dst = vv[:, st, :].rearrange("p (h x) -> p h x", h=NH)
                    nc.scalar.activation(
                        out=dst[:, :, 0:DEP],
                        in_=ps_v[st].rearrange("p (h x) -> p h x", h=NH),
                        func=AF.Copy, scale=1.0)
                    nc.vector.tensor_copy(out=dst[:, :, DEP:DEP + 1], in_=ones_sb)

            # A1: qq^T, kk^T   (psum [128,1024] x2 each = 8 banks)
            with tc.tile_pool(name="psA1", bufs=4, space="PSUM") as psA1:
                ps_q = [psA1.tile([128, S], F, tag="psA1", name=f"psq{i}") for i in range(2)]
                ps_k = [psA1.tile([128, S], F, tag="psA1", name=f"psk{i}") for i in range(2)]
                for kt in range(KT):
                    rows = slice(kt * 128, (kt + 1) * 128)
                    nc.sync.dma_start(out=wq_s[:, kt, :], in_=d["wq"][rows, :])
                    nc.sync.dma_start(out=wk_s[:, kt, :], in_=d["wk"][rows, :])
                    qin = pin.tile([128, S], FR, tag="pin")
                    nc.sync.dma_start(out=qin, in_=d["qT"][rows, :])
                    kin = pin.tile([128, S], FR, tag="pin")
                    nc.sync.dma_start(out=kin, in_=d["kT"][rows, :])
                    for dc in range(2):
                        for cc in range(2):
                            sl = slice(cc * 512, (cc + 1) * 512)
                            nc.tensor.matmul(
                                ps_q[dc][:, sl],
                                wq_s[:, kt, dc * 128:(dc + 1) * 128],
                                qin[:, sl],
                                start=(kt == 0), stop=(kt == KT - 1))
                            nc.tensor.matmul(
                                ps_k[dc][:, sl],
                                wk_s[:, kt, dc * 128:(dc + 1) * 128],
                                kin[:, sl],
                                start=(kt == 0), stop=(kt == KT - 1))
                # mask + wo loads land behind the q/k stream (needed later)
                for kt in range(KT):
                    nc.sync.dma_start(
                        out=keep_s[:, kt, :],
                        in_=d["keepT"][kt * 128:(kt + 1) * 128, :])
                for h in range(NH):
                    nc.sync.dma_start(
                        out=wo_s[:, h, :], in_=d["wo"][h * DEP:(h + 1) * DEP, :])
                # evacuate with bias add (Identity allows per-partition bias AP)
                for dc in range(2):
                    for hf in range(2):
                        h = dc * 2 + hf
                        rows = slice(hf * DEP, (hf + 1) * DEP)
                        nc.scalar.activation(
                            out=qqT[:, h, :], in_=ps_q[dc][rows, :],
                            func=AF.Identity, bias=bq_s[:, h:h + 1], scale=1.0)
                        nc.scalar.activation(
                            out=kkT[:, h, :], in_=ps_k[dc][rows, :],
                            func=AF.Identity, bias=bk_s[:, h:h + 1], scale=1.0)

        # ---------------- phase B: heads ----------------
        with tc.tile_pool(name="escore", bufs=18) as epool, \
             tc.tile_pool(name="utile", bufs=3) as upool, \
             tc.tile_pool(name="attsb", bufs=4) as apool, \
             tc.tile_pool(name="rtiles", bufs=2) as rpool, \
             tc.tile_pool(name="psS", bufs=2, space="PSUM") as psS, \
             tc.tile_pool(name="psAV", bufs=2, space="PSUM") as psAV:
            for h in range(NH):
                e_tiles = []
                av = psAV.tile([65, S], F, tag="av")
                for kt in range(KT):
                    ps = psS.tile([128, S], F, tag="score")
                    for cc in range(2):
                        sl = slice(cc * 512, (cc + 1) * 512)
                        nc.tensor.matmul(
                            ps[:, sl],
                            kkT[:, h, kt * 128:(kt + 1) * 128],
                            qqT[:, h, sl],
                            start=True, stop=True)
                    u = upool.tile([128, S], F16, tag="u")
                    nc.scalar.activation(out=u, in_=ps, func=AF.Exp, scale=0.125)
                    e = epool.tile([128, S], F16, tag="e")
                    nc.vector.scalar_tensor_tensor(
                        out=e, in0=u, scalar=1.0, in1=keep_s[:, kt, :],
                        op0=OP.max, op1=OP.mult)
                    e_tiles.append(e)
                    for cc in range(2):
                        sl = slice(cc * 512, (cc + 1) * 512)
                        nc.tensor.matmul(
                            av[:, sl],
                            vv[:, kt, h * 65:(h + 1) * 65],
                            e[:, sl],
                            start=(kt == 0), stop=(kt == KT - 1))
                # reciprocal of sums, broadcast to all partitions
                srow = rpool.tile([1, S], F, tag="srow")
                nc.scalar.activation(out=srow, in_=av[64:65, :], func=AF.Copy, scale=1.0)
                sbc = rpool.tile([128, S], F, tag="sbc")
                nc.gpsimd.partition_broadcast(sbc, srow)
                rb = rpool.tile([128, S], F, tag="rb")
                nc.vector.reciprocal_approx_fast(out=rb, in_=sbc)
                rbh = rpool.tile([128, S], F16, tag="rbh")
                nc.vector.tensor_copy(out=rbh, in_=rb)
                # normalized attention out (att^T layout), split DVE/GPSIMD
                for kt in range(KT):
                    att_sb = apool.tile([128, S], F16, tag="att")
                    eng = nc.vector if kt % 2 == 0 else nc.gpsimd
                    eng.tensor_tensor(
                        out=att_sb, in0=e_tiles[kt], in1=rbh, op=OP.mult)
                    nc.sync.dma_start(out=d["attT"][h, kt * 128:(kt + 1) * 128, :], in_=att_sb)
                # normalized av^T
                nc.vector.tensor_tensor(
                    out=oav[h], in0=av[0:DEP, :], in1=rb[0:DEP, :], op=OP.mult)
                if _DEBUG:
                    av_dump = apool.tile([65, S], F, tag="avdump", name=f"avd{h}")
                    nc.scalar.activation(out=av_dump, in_=av, func=AF.Copy, scale=1.0)
                    nc.sync.dma_start(out=d["dbg_av"][h], in_=av_dump)
                    nc.sync.dma_start(out=d["dbg_oav"][h], in_=oav[h].bitcast(F))

        # ---------------- phase C: output projection ----------------
        with tc.tile_pool(name="osb", bufs=3) as opool, \
             tc.tile_pool(name="psO", bufs=4, space="PSUM") as psO:
            for qt in range(NT):
                out_sb = opool.tile([128, DIN], F, tag="osb")
                for nch in range(2):
                    po = psO.tile([128, 512], F, tag="po")
                    for h in range(NH):
                        nc.tensor.matmul(
                            po,
                            oav[h][:, qt * 128:(qt + 1) * 128],
                            wo_s[:, h, nch * 512:(nch + 1) * 512],
                            start=(h == 0), stop=(h == NH - 1))
                    nc.scalar.activation(
                        out=out_sb[:, nch * 512:(nch + 1) * 512], in_=po,
                        func=AF.Copy, scale=1.0)
                nc.sync.dma_start(out=d["outp"][qt * 128:(qt + 1) * 128, :], in_=out_sb)


def _get_nc():
    if "nc" not in _CACHE:
        _CACHE["nc"] = _build()
    return _CACHE["nc"]


def kernel(v, k, q, mask, Wq0, bq0, Wk0, bk0, Wv, bv, Wo, bo):
    v = np.asarray(v, dtype=np.float32)
    k = np.asarray(k, dtype=np.float32)
    q = np.asarray(q, dtype=np.float32)
    mask = np.asarray(mask)
    Wq0 = np.asarray(Wq0, dtype=np.float32)
    Wk0 = np.asarray(Wk0, dtype=np.float32)
    Wv = np.asarray(Wv, dtype=np.float32)
    Wo = np.asarray(Wo, dtype=np.float32)
    bq0 = np.asarray(bq0, dtype=np.float32)
    bk0 = np.asarray(bk0, dtype=np.float32)
    bv = np.asarray(bv, dtype=np.float32)
    bo = np.asarray(bo, dtype=np.float32)
    B = v.shape[0]
    HTOT = 16

    nc = _get_nc()

    per_batch = []
    for b in range(B):
        per_batch.append({
            "qT": np.ascontiguousarray(q[b, 1:, :].T),
            "kT": np.ascontiguousarray(k[b, :-1, :].T),
            "vT": np.ascontiguousarray(v[b].T),
            "keepT": np.ascontiguousarray((1 - mask[b]).T).astype(np.float16),
        })
    in_maps = []
    for c in range(8):
        b, g = c // 4, c % 4
        cols = slice(g * DCOL, (g + 1) * DCOL)
        m = dict(per_batch[b])
        m["wq"] = np.ascontiguousarray(Wq0[:, cols])
        m["wk"] = np.ascontiguousarray(Wk0[:, cols])
        m["wv"] = np.ascontiguousarray(Wv[:, cols])
        m["wo"] = np.ascontiguousarray(Wo[cols, :])
        m["bq"] = np.ascontiguousarray(bq0[cols].reshape(NH, DEP).T)
        m["bk"] = np.ascontiguousarray(bk0[cols].reshape(NH, DEP).T)
        in_maps.append(m)

    res = run_bass_kernel_spmd(nc, in_maps, core_ids=list(range(8)))

    att = np.empty((B, HTOT, S, S), dtype=np.float64)
    out = np.empty((B, S, DIN), dtype=np.float64)
    bias_row = (bv.astype(np.float64) @ Wo.astype(np.float64)) + bo.astype(np.float64)
    for b in range(B):
        acc = None
        for g in range(4):
            r = res.results[b * 4 + g]
            attT = r["attT"]
            for hl in range(NH):
                att[b, g * NH + hl] = attT[hl].T
            acc = r["outp"].astype(np.float64) if acc is None else acc + r["outp"]
        out[b] = acc + bias_row[None, :]
    return out, att


# revision 16
# speedup vs baseline: 1.1835x; 1.0504x over previous
"""Trainium2 Bass kernel for nn_MultiHeadAttention_67757404062370.

Sharding: data-parallel over batch (2) x tensor-parallel over heads (4 groups
of 4 heads) = 8 NeuronCores. Core c handles batch c//4, heads 4*(c%4)..4*(c%4)+3.

Device-side math (per core, everything in "transposed" layout):
  vv   = v Wv_g  (fp32r matmul)               [seq, dcol] + ones cols, fp16
  qq^T = Wq_g^T q^T (+bq), kk^T likewise      [dcol, seq], fp32r
  s^T[k,q]  = lhsT=kh^T tile, rhs=qh^T        (K=64, fp32r)
  u = exp(s/8) fp16; e = max(u,1)*keep fp16   (exp(relu(x)) == max(exp(x),1))
  av^T[c,q] (+ sums row via ones column) = [vv|1]^T @ e   (fp16 matmul)
  att^T = e * (1/sums) fp16 ; oav^T = av^T * (1/sums) fp32r
  outp[q,:] = sum_h oav_h^T.T @ Wo_rows_h  (K=64 per head, fp32r)
Host: gather, transpose att views, sum outp over the 4 head-group cores per
batch, add (bv @ Wo + bo), cast to float64.
"""

import sys

if "/opt/trn_rl_repo" not in sys.path:
    sys.path.insert(0, "/opt/trn_rl_repo")

import numpy as np
import ml_dtypes

import concourse.bacc as bacc
import concourse.tile as tile
from concourse import mybir
from concourse.bass_utils import run_bass_kernel_spmd

F = mybir.dt.float32
FR = mybir.dt.float32r
F16 = mybir.dt.float16
AF = mybir.ActivationFunctionType
OP = mybir.AluOpType

S = 1024      # seq len (post-slice) == v seq len
DIN = 1024    # d_model
NH = 4        # heads per core
DEP = 64      # head depth
DCOL = NH * DEP  # 256 projection columns per core
NT = S // 128    # 8 seq tiles
KT = DIN // 128  # 8 contraction tiles

_CACHE = {}
_DEBUG = False


def _build():
    nc = bacc.Bacc("TRN2", target_bir_lowering=False, debug=False, num_devices=8)

    d = {}
    d["qT"] = nc.dram_tensor("qT", [DIN, S], FR, kind="ExternalInput").ap()
    d["kT"] = nc.dram_tensor("kT", [DIN, S], FR, kind="ExternalInput").ap()
    d["vT"] = nc.dram_tensor("vT", [DIN, S], FR, kind="ExternalInput").ap()
    d["keepT"] = nc.dram_tensor("keepT", [S, S], F16, kind="ExternalInput").ap()
    d["wq"] = nc.dram_tensor("wq", [DIN, DCOL], FR, kind="ExternalInput").ap()
    d["wk"] = nc.dram_tensor("wk", [DIN, DCOL], FR, kind="ExternalInput").ap()
    d["wv"] = nc.dram_tensor("wv", [DIN, DCOL], FR, kind="ExternalInput").ap()
    d["wo"] = nc.dram_tensor("wo", [DCOL, DIN], FR, kind="ExternalInput").ap()
    d["bq"] = nc.dram_tensor("bq", [DEP, NH], F, kind="ExternalInput").ap()
    d["bk"] = nc.dram_tensor("bk", [DEP, NH], F, kind="ExternalInput").ap()
    d["attT"] = nc.dram_tensor("attT", [NH, S, S], F16, kind="ExternalOutput").ap()
    d["outp"] = nc.dram_tensor("outp", [S, DIN], F, kind="ExternalOutput").ap()
    if _DEBUG:
        d["dbg_av"] = nc.dram_tensor("dbg_av", [NH, 65, S], F, kind="ExternalOutput").ap()
        d["dbg_oav"] = nc.dram_tensor("dbg_oav", [NH, DEP, S], F, kind="ExternalOutput").ap()

    with tile.TileContext(nc) as tc:
        _emit(nc, tc, d)
    nc.compile()
    return nc


def _emit(nc, tc, d):
    from contextlib import ExitStack

    ctx = ExitStack()
    with ctx:
        # ---------------- persistent tiles ----------------
        persist = ctx.enter_context(tc.tile_pool(name="persist", bufs=1))
        qqT = persist.tile([DEP, NH, S], FR, tag="qqT")
        kkT = persist.tile([DEP, NH, S], FR, tag="kkT")
        # v projection w/ ones column: [kseq-in-tile, ktile, head*65], fp16
        vv = persist.tile([128, NT, NH * 65], F16, tag="vv")
        wo_s = persist.tile([DEP, NH, DIN], FR, tag="wo")
        keep_s = persist.tile([128, KT, S], F16, tag="keep")
        bq_s = persist.tile([DEP, NH], F, tag="bq")
        bk_s = persist.tile([DEP, NH], F, tag="bk")
        ones_sb = persist.tile([128, NH, 1], F, tag="ones")
        nc.vector.memset(ones_sb, 1.0)
        oav = [persist.tile([DEP, S], FR, tag=f"oav{h}", name=f"oav{h}") for h in range(NH)]

        nc.sync.dma_start(out=bq_s, in_=d["bq"])
        nc.sync.dma_start(out=bk_s, in_=d["bk"])

        # ---------------- phase A: projections ----------------
        # v first: its PSUM banks (8) free up before q/k accumulation needs
        # them, and vv is ready early for the first head's AV matmuls.
        with tc.tile_pool(name="wqkv", bufs=1) as wpool, \
             tc.tile_pool(name="projin", bufs=4) as pin:
            wq_s = wpool.tile([128, KT, DCOL], FR, tag="wq")
            wk_s = wpool.tile([128, KT, DCOL], FR, tag="wk")
            wv_s = wpool.tile([128, KT, DCOL], FR, tag="wv")

            # A2: vv (normal layout) + ones columns
            # NB: matmul start=True clears the whole PSUM bank, so every
            # accumulation group needs its own bank (8 tiles of [128, 256]).
            with tc.tile_pool(name="psA2", bufs=1, space="PSUM") as psA2:
                ps_v = [psA2.tile([128, 256], F, tag=f"psA2{i}", name=f"psv{i}") for i in range(NT)]
                for kt in range(KT):
                    rows = slice(kt * 128, (kt + 1) * 128)
                    nc.sync.dma_start(out=wv_s[:, kt, :], in_=d["wv"][rows, :])
                    vin = pin.tile([128, S], FR, tag="pin")
                    nc.sync.dma_start(out=vin, in_=d["vT"][rows, :])
                    for st in range(NT):
                        nc.tensor.matmul(
                            ps_v[st],
                            vin[:, st * 128:(st + 1) * 128],
                            wv_s[:, kt, :],
                            start=(kt == 0), stop=(kt == KT - 1))
                for st in range(NT):
                    dst = vv[:, st, :].rearrange("p (h x) -> p h x", h=NH)
                    nc.scalar.activation(
                        out=dst[:, :, 0:DEP],
                        in_=ps_v[st].rearrange("p (h x) -> p h x", h=NH),
                        func=AF.Copy, scale=1.0)
                    nc.vector.tensor_copy(out=dst[:, :, DEP:DEP + 1], in_=ones_sb)

            # A1: qq^T, kk^T   (psum [128,1024] x2 each = 8 banks)
            with tc.tile_pool(name="psA1", bufs=4, space="PSUM") as psA1:
                ps_q = [psA1.tile([128, S], F, tag="psA1", name=f"psq{i}") for i in range(2)]
                ps_k = [psA1.tile([128, S], F, tag="psA1", name=f"psk{i}") for i in range(2)]
                for kt in range(KT):
                    rows = slice(kt * 128, (kt + 1) * 128)
                    nc.sync.dma_start(out=wq_s[:, kt, :], in_=d["wq"][rows, :])
                    nc.sync.dma_start(out=wk_s[:, kt, :], in_=d["wk"][rows, :])
                    qin = pin.tile([128, S], FR, tag="pin")
                    nc.sync.dma_start(out=qin, in_=d["qT"][rows, :])
                    kin = pin.tile([128, S], FR, tag="pin")
                    nc.sync.dma_start(out=kin, in_=d["kT"][rows, :])
                    for dc in range(2):
                        for cc in range(2):
                            sl = slice(cc * 512, (cc + 1) * 512)
                            nc.tensor.matmul(
                                ps_q[dc][:, sl],
                                wq_s[:, kt, dc * 128:(dc + 1) * 128],
                                qin[:, sl],
                                start=(kt == 0), stop=(kt == KT - 1))
                            nc.tensor.matmul(
                                ps_k[dc][:, sl],
                                wk_s[:, kt, dc * 128:(dc + 1) * 128],
                                kin[:, sl],
                                start=(kt == 0), stop=(kt == KT - 1))
                # mask + wo loads land behind the q/k stream (needed later)
                for kt in range(KT):
                    nc.sync.dma_start(
                        out=keep_s[:, kt, :],
                        in_=d["keepT"][kt * 128:(kt + 1) * 128, :])
                for h in range(NH):
                    nc.sync.dma_start(
                        out=wo_s[:, h, :], in_=d["wo"][h * DEP:(h + 1) * DEP, :])
                # evacuate with bias add (Identity allows per-partition bias AP)
                for dc in range(2):
                    for hf in range(2):
                        h = dc * 2 + hf
                        rows = slice(hf * DEP, (hf + 1) * DEP)
                        nc.scalar.activation(
                            out=qqT[:, h, :], in_=ps_q[dc][rows, :],
                            func=AF.Identity, bias=bq_s[:, h:h + 1], scale=1.0)
                        nc.scalar.activation(
                            out=kkT[:, h, :], in_=ps_k[dc][rows, :],
                            func=AF.Identity, bias=bk_s[:, h:h + 1], scale=1.0)

        # ---------------- phase B: heads (processed in pairs) ----------------
        # Two heads interleave so the PE fills one chain's exp/mask stalls
        # with the other head's scores; PSUM = 2 scores tiles + 2 av = 8 banks.
        with tc.tile_pool(name="escore", bufs=18) as epool, \
             tc.tile_pool(name="utile", bufs=4) as upool, \
             tc.tile_pool(name="attsb", bufs=6) as apool, \
             tc.tile_pool(name="rtiles", bufs=2) as rpool, \
             tc.tile_pool(name="psS", bufs=2, space="PSUM") as psS, \
             tc.tile_pool(name="psAV", bufs=2, space="PSUM") as psAV:
            for hp in range(NH // 2):
                heads = (2 * hp, 2 * hp + 1)
                e_tiles = {h: [] for h in heads}
                av = {h: psAV.tile([65, S], F, tag="av", name=f"av{h}") for h in heads}
                for kt in range(KT):
                    for h in heads:
                        ps = psS.tile([128, S], F, tag="score", name=f"sc{h}_{kt}")
                        for cc in range(2):
                            sl = slice(cc * 512, (cc + 1) * 512)
                            nc.tensor.matmul(
                                ps[:, sl],
                                kkT[:, h, kt * 128:(kt + 1) * 128],
                                qqT[:, h, sl],
                                start=True, stop=True)
                        u = upool.tile([128, S], F16, tag="u", name=f"u{h}_{kt}")
                        nc.scalar.activation(out=u, in_=ps, func=AF.Exp, scale=0.125)
                        t = upool.tile([128, S], F16, tag="t", name=f"t{h}_{kt}")
                        nc.vector.tensor_scalar_max(out=t, in0=u, scalar1=1.0)
                        e = epool.tile([128, S], F16, tag="e", name=f"e{h}_{kt}")
                        nc.vector.tensor_tensor(
                            out=e, in0=t, in1=keep_s[:, kt, :], op=OP.mult)
                        e_tiles[h].append(e)
                        for cc in range(2):
                            sl = slice(cc * 512, (cc + 1) * 512)
                            nc.tensor.matmul(
                                av[h][:, sl],
                                vv[:, kt, h * 65:(h + 1) * 65],
                                e[:, sl],
                                start=(kt == 0), stop=(kt == KT - 1))
                for h in heads:
                    # reciprocal of sums, broadcast to all partitions
                    srow = rpool.tile([1, S], F, tag="srow", name=f"sr{h}")
                    nc.scalar.activation(out=srow, in_=av[h][64:65, :], func=AF.Copy, scale=1.0)
                    sbc = rpool.tile([128, S], F, tag="sbc", name=f"sb{h}")
                    nc.gpsimd.partition_broadcast(sbc, srow)
                    rb = rpool.tile([128, S], F, tag="rb", name=f"rb{h}")
                    nc.vector.reciprocal_approx_fast(out=rb, in_=sbc)
                    rbh = rpool.tile([128, S], F16, tag="rbh", name=f"rh{h}")
                    nc.vector.tensor_copy(out=rbh, in_=rb)
                    # normalized attention out (att^T layout), split DVE/GPSIMD
                    for kt in range(KT):
                        att_sb = apool.tile([128, S], F16, tag="att", name=f"at{h}_{kt}")
                        eng = nc.vector if kt % 4 != 3 else nc.gpsimd
                        eng.tensor_tensor(
                            out=att_sb, in0=e_tiles[h][kt], in1=rbh, op=OP.mult)
                        nc.sync.dma_start(out=d["attT"][h, kt * 128:(kt + 1) * 128, :], in_=att_sb)
                    # normalized av^T
                    nc.vector.tensor_tensor(
                        out=oav[h], in0=av[h][0:DEP, :], in1=rb[0:DEP, :], op=OP.mult)
                    if _DEBUG:
                        av_dump = apool.tile([65, S], F, tag="avdump", name=f"avd{h}")
                        nc.scalar.activation(out=av_dump, in_=av[h], func=AF.Copy, scale=1.0)
                        nc.sync.dma_start(out=d["dbg_av"][h], in_=av_dump)
                        nc.sync.dma_start(out=d["dbg_oav"][h], in_=oav[h].bitcast(F))

        # ---------------- phase C: output projection ----------------
        with tc.tile_pool(name="osb", bufs=3) as opool, \
             tc.tile_pool(name="psO", bufs=4, space="PSUM") as psO:
            for qt in range(NT):
                out_sb = opool.tile([128, DIN], F, tag="osb")
                for nch in range(2):
                    po = psO.tile([128, 512], F, tag="po")
                    for h in range(NH):
                        nc.tensor.matmul(
                            po,
                            oav[h][:, qt * 128:(qt + 1) * 128],
                            wo_s[:, h, nch * 512:(nch + 1) * 512],
                            start=(h == 0), stop=(h == NH - 1))
                    nc.scalar.activation(
                        out=out_sb[:, nch * 512:(nch + 1) * 512], in_=po,
                        func=AF.Copy, scale=1.0)
                nc.sync.dma_start(out=d["outp"][qt * 128:(qt + 1) * 128, :], in_=out_sb)


def _get_nc():
    if "nc" not in _CACHE:
        _CACHE["nc"] = _build()
    return _CACHE["nc"]


def kernel(v, k, q, mask, Wq0, bq0, Wk0, bk0, Wv, bv, Wo, bo):
    v = np.asarray(v, dtype=np.float32)
    k = np.asarray(k, dtype=np.float32)
    q = np.asarray(q, dtype=np.float32)
    mask = np.asarray(mask)
    Wq0 = np.asarray(Wq0, dtype=np.float32)
    Wk0 = np.asarray(Wk0, dtype=np.float32)
    Wv = np.asarray(Wv, dtype=np.float32)
    Wo = np.asarray(Wo, dtype=np.float32)
    bq0 = np.asarray(bq0, dtype=np.float32)
    bk0 = np.asarray(bk0, dtype=np.float32)
    bv = np.asarray(bv, dtype=np.float32)
    bo = np.asarray(bo, dtype=np.float32)
    B = v.shape[0]
    HTOT = 16

    nc = _get_nc()

    per_batch = []
    for b in range(B):
        per_batch.append({
            "qT": np.ascontiguousarray(q[b, 1:, :].T),
            "kT": np.ascontiguousarray(k[b, :-1, :].T),
            "vT": np.ascontiguousarray(v[b].T),
            "keepT": np.ascontiguousarray((1 - mask[b]).T).astype(np.float16),
        })
    in_maps = []
    for c in range(8):
        b, g = c // 4, c % 4
        cols = slice(g * DCOL, (g + 1) * DCOL)
        m = dict(per_batch[b])
        m["wq"] = np.ascontiguousarray(Wq0[:, cols])
        m["wk"] = np.ascontiguousarray(Wk0[:, cols])
        m["wv"] = np.ascontiguousarray(Wv[:, cols])
        m["wo"] = np.ascontiguousarray(Wo[cols, :])
        m["bq"] = np.ascontiguousarray(bq0[cols].reshape(NH, DEP).T)
        m["bk"] = np.ascontiguousarray(bk0[cols].reshape(NH, DEP).T)
        in_maps.append(m)

    res = run_bass_kernel_spmd(nc, in_maps, core_ids=list(range(8)))

    att = np.empty((B, HTOT, S, S), dtype=np.float64)
    out = np.empty((B, S, DIN), dtype=np.float64)
    bias_row = (bv.astype(np.float64) @ Wo.astype(np.float64)) + bo.astype(np.float64)
    for b in range(B):
        acc = None
        for g in range(4):
            r = res.results[b * 4 + g]
            attT = r["attT"]
            for hl in range(NH):
                att[b, g * NH + hl] = attT[hl].T
            acc = r["outp"].astype(np.float64) if acc is None else acc + r["outp"]
        out[b] = acc + bias_row[None, :]
    return out, att


# revision 18
# speedup vs baseline: 1.3324x; 1.1258x over previous
"""Trainium2 Bass kernel for nn_MultiHeadAttention_67757404062370.

Sharding: data-parallel over batch (2) x tensor-parallel over heads (4 groups
of 4 heads) = 8 NeuronCores. Core c handles batch c//4, heads 4*(c%4)..4*(c%4)+3.

Device-side per core (transposed layout throughout):
  kk^T = Wk_g^T k^T (+bk)  [128=2 heads x 64, 2, seq] fp32r
  qq^T = Wq_g^T q^T (+bq)  [128, 4, seq] fp32r, zero-padded in the other
         head's 64 rows so the scores contraction can use K=128 (K=64
         matmuls never warm the PE clock gate).
  vv   = v Wv_g  [seq, 4*65] fp16 with ones columns
  s^T[k,q] = kkT_tile.T @ qqT_pad   (K=128, fp32r)
  u = exp(s/8) fp16; t = max(u,1); e = t*keep  (exp(relu(x)) == max(exp(x),1))
  av^T (+ sums row via ones cols) = vv_aug.T @ e   (fp16, K=128)
  att^T = e * (1/sums) fp16 ; oav^T = av^T * (1/sums) fp32r
  outp[q,:] = sum_h oav_h^T.T @ Wo_rows_h  (K=64 per head, fp32r)
Host: gather, transpose att views, sum outp over the 4 head-group cores per
batch, add (bv @ Wo + bo), cast to float64.
"""

import sys

if "/opt/trn_rl_repo" not in sys.path:
    sys.path.insert(0, "/opt/trn_rl_repo")

import numpy as np

import concourse.bacc as bacc
import concourse.tile as tile
from concourse import mybir
from concourse.bass_utils import run_bass_kernel_spmd

F = mybir.dt.float32
FR = mybir.dt.float32r
F16 = mybir.dt.float16
AF = mybir.ActivationFunctionType
OP = mybir.AluOpType

S = 1024
DIN = 1024
NH = 4
DEP = 64
DCOL = NH * DEP
NT = S // 128
KT = DIN // 128

_CACHE = {}
_DEBUG = False


def _build():
    nc = bacc.Bacc("TRN2", target_bir_lowering=False, debug=False, num_devices=8)

    d = {}
    d["qT"] = nc.dram_tensor("qT", [DIN, S], FR, kind="ExternalInput").ap()
    d["kT"] = nc.dram_tensor("kT", [DIN, S], FR, kind="ExternalInput").ap()
    d["vT"] = nc.dram_tensor("vT", [DIN, S], F16, kind="ExternalInput").ap()
    d["keepT"] = nc.dram_tensor("keepT", [S, S], F16, kind="ExternalInput").ap()
    d["wq"] = nc.dram_tensor("wq", [DIN, DCOL], FR, kind="ExternalInput").ap()
    d["wk"] = nc.dram_tensor("wk", [DIN, DCOL], FR, kind="ExternalInput").ap()
    d["wv"] = nc.dram_tensor("wv", [DIN, DCOL], F16, kind="ExternalInput").ap()
    d["wo"] = nc.dram_tensor("wo", [DCOL, DIN], FR, kind="ExternalInput").ap()
    d["bq"] = nc.dram_tensor("bq", [DEP, NH], F, kind="ExternalInput").ap()
    d["bk"] = nc.dram_tensor("bk", [128, 2], F, kind="ExternalInput").ap()
    d["attT"] = nc.dram_tensor("attT", [NH, S, S], F16, kind="ExternalOutput").ap()
    d["outp"] = nc.dram_tensor("outp", [S, DIN], F, kind="ExternalOutput").ap()
    if _DEBUG:
        d["dbg_av"] = nc.dram_tensor("dbg_av", [NH, 65, S], F, kind="ExternalOutput").ap()
        d["dbg_oav"] = nc.dram_tensor("dbg_oav", [NH, DEP, S], F, kind="ExternalOutput").ap()

    with tile.TileContext(nc) as tc:
        _emit(nc, tc, d)
    nc.compile()
    return nc


def _emit(nc, tc, d):
    from contextlib import ExitStack

    ctx = ExitStack()
    with ctx:
        # ---------------- persistent tiles ----------------
        persist = ctx.enter_context(tc.tile_pool(name="persist", bufs=1))
        # q proj, zero-padded per head to a full 128-row contraction
        qqT = persist.tile([128, NH, S], FR, tag="qqT")
        # k proj, two heads stacked per dcol-tile
        kkT = persist.tile([128, 2, S], FR, tag="kkT")
        vv = persist.tile([128, NT, NH * 65], F16, tag="vv")
        wo_s = persist.tile([DEP, NH, DIN], FR, tag="wo")
        keep_s = persist.tile([128, KT, S], F16, tag="keep")
        bq_s = persist.tile([DEP, NH], F, tag="bq")
        bk_s = persist.tile([128, 2], F, tag="bk")
        ones_sb = persist.tile([128, NH, 1], F, tag="ones")
        nc.vector.memset(ones_sb, 1.0)
        oav = [persist.tile([DEP, S], FR, tag=f"oav{h}", name=f"oav{h}") for h in range(NH)]

        nc.sync.dma_start(out=bq_s, in_=d["bq"])
        nc.sync.dma_start(out=bk_s, in_=d["bk"])

        # ---------------- phase A: q/k projections ----------------
        with tc.tile_pool(name="wqk", bufs=1) as wpool, \
             tc.tile_pool(name="projin", bufs=4) as pin:
            wq_s = wpool.tile([128, KT, DCOL], FR, tag="wq")
            wk_s = wpool.tile([128, KT, DCOL], FR, tag="wk")
            with tc.tile_pool(name="psA1", bufs=4, space="PSUM") as psA1:
                ps_q = [psA1.tile([128, S], F, tag="psA1", name=f"psq{i}") for i in range(2)]
                ps_k = [psA1.tile([128, S], F, tag="psA1", name=f"psk{i}") for i in range(2)]
                for kt in range(KT):
                    rows = slice(kt * 128, (kt + 1) * 128)
                    nc.sync.dma_start(out=wq_s[:, kt, :], in_=d["wq"][rows, :])
                    nc.sync.dma_start(out=wk_s[:, kt, :], in_=d["wk"][rows, :])
                    qin = pin.tile([128, S], FR, tag="pin")
                    nc.sync.dma_start(out=qin, in_=d["qT"][rows, :])
                    kin = pin.tile([128, S], FR, tag="pin")
                    nc.sync.dma_start(out=kin, in_=d["kT"][rows, :])
                    for dc in range(2):
                        for cc in range(2):
                            sl = slice(cc * 512, (cc + 1) * 512)
                            nc.tensor.matmul(
                                ps_q[dc][:, sl],
                                wq_s[:, kt, dc * 128:(dc + 1) * 128],
                                qin[:, sl],
                                start=(kt == 0), stop=(kt == KT - 1))
                            nc.tensor.matmul(
                                ps_k[dc][:, sl],
                                wk_s[:, kt, dc * 128:(dc + 1) * 128],
                                kin[:, sl],
                                start=(kt == 0), stop=(kt == KT - 1))
                # remaining input loads queue behind the q/k stream
                for kt in range(KT):
                    nc.sync.dma_start(
                        out=keep_s[:, kt, :],
                        in_=d["keepT"][kt * 128:(kt + 1) * 128, :])
                # evacuate: kk full tiles w/ bias; qq per head w/ zero padding
                for dc in range(2):
                    nc.scalar.activation(
                        out=kkT[:, dc, :], in_=ps_k[dc],
                        func=AF.Identity, bias=bk_s[:, dc:dc + 1], scale=1.0)
                    for hf in range(2):
                        h = dc * 2 + hf
                        rw = slice(hf * DEP, (hf + 1) * DEP)
                        zw = slice((1 - hf) * DEP, (2 - hf) * DEP)
                        nc.scalar.activation(
                            out=qqT[rw, h, :], in_=ps_q[dc][rw, :],
                            func=AF.Identity, bias=bq_s[:, h:h + 1], scale=1.0)
                        # zero rows: Identity(0*x + 0)
                        nc.scalar.activation(
                            out=qqT[zw, h, :], in_=ps_q[dc][zw, :],
                            func=AF.Identity, bias=0.0, scale=0.0)

        # ---------------- v input stream (fp16) ----------------
        with tc.tile_pool(name="wvp", bufs=1) as wvpool, \
             tc.tile_pool(name="vtp", bufs=1) as vtp:
            wv_s = wvpool.tile([128, KT, DCOL], F16, tag="wv")
            vt_s = [vtp.tile([128, S], F16, tag=f"vt{i}", name=f"vt{i}") for i in range(KT)]
            for kt in range(KT):
                rows = slice(kt * 128, (kt + 1) * 128)
                nc.sync.dma_start(out=wv_s[:, kt, :], in_=d["wv"][rows, :])
                nc.sync.dma_start(out=vt_s[kt], in_=d["vT"][rows, :])
            for h in range(NH):
                nc.sync.dma_start(
                    out=wo_s[:, h, :], in_=d["wo"][h * DEP:(h + 1) * DEP, :])

            # ---------------- phase B: heads in pairs ----------------
            with tc.tile_pool(name="escore", bufs=18) as epool, \
                 tc.tile_pool(name="utile", bufs=3) as upool, \
                 tc.tile_pool(name="attsb", bufs=4) as apool, \
                 tc.tile_pool(name="rtiles", bufs=2) as rpool, \
                 tc.tile_pool(name="psS", bufs=2, space="PSUM") as psS, \
                 tc.tile_pool(name="psAV", bufs=2, space="PSUM") as psAV:
                for hp in range(NH // 2):
                    heads = (2 * hp, 2 * hp + 1)
                    e_tiles = {h: [] for h in heads}
                    # scores + exp + mask for both heads of the pair
                    for kt in range(KT):
                        for h in heads:
                            ps = psS.tile([128, S], F, tag="score", name=f"sc{h}_{kt}")
                            for cc in range(2):
                                sl = slice(cc * 512, (cc + 1) * 512)
                                nc.tensor.matmul(
                                    ps[:, sl],
                                    kkT[:, hp, kt * 128:(kt + 1) * 128],
                                    qqT[:, h, sl],
                                    start=True, stop=True)
                            u = upool.tile([128, S], F16, tag="u", name=f"u{h}_{kt}")
                            nc.scalar.activation(out=u, in_=ps, func=AF.Exp, scale=0.125)
                            t = upool.tile([128, S], F16, tag="t", name=f"t{h}_{kt}")
                            nc.vector.tensor_scalar_max(out=t, in0=u, scalar1=1.0)
                            e = epool.tile([128, S], F16, tag="e", name=f"e{h}_{kt}")
                            eng = nc.gpsimd if kt in (2, 5) else nc.vector
                            eng.tensor_tensor(
                                out=e, in0=t, in1=keep_s[:, kt, :], op=OP.mult)
                            e_tiles[h].append(e)
                    if hp == 0:
                        # vv projection: sequential per seq-tile, slots shared
                        # with the scores tag (PE reaches here after S0)
                        for st in range(NT):
                            pv = psS.tile([128, 256], F, tag="score", name=f"psv{st}")
                            for kt in range(KT):
                                nc.tensor.matmul(
                                    pv,
                                    vt_s[kt][:, st * 128:(st + 1) * 128],
                                    wv_s[:, kt, :],
                                    start=(kt == 0), stop=(kt == KT - 1))
                            dst = vv[:, st, :].rearrange("p (h x) -> p h x", h=NH)
                            nc.scalar.activation(
                                out=dst[:, :, 0:DEP],
                                in_=pv.rearrange("p (h x) -> p h x", h=NH),
                                func=AF.Copy, scale=1.0)
                            nc.vector.tensor_copy(out=dst[:, :, DEP:DEP + 1], in_=ones_sb)
                    # AV matmuls per head
                    av = {}
                    for h in heads:
                        av[h] = psAV.tile([65, S], F, tag="av", name=f"av{h}")
                        for kt in range(KT):
                            for cc in range(2):
                                sl = slice(cc * 512, (cc + 1) * 512)
                                nc.tensor.matmul(
                                    av[h][:, sl],
                                    vv[:, kt, h * 65:(h + 1) * 65],
                                    e_tiles[h][kt][:, sl],
                                    start=(kt == 0), stop=(kt == KT - 1))
                    for h in heads:
                        srow = rpool.tile([1, S], F, tag="srow", name=f"sr{h}")
                        nc.scalar.activation(out=srow, in_=av[h][64:65, :], func=AF.Copy, scale=1.0)
                        sbc = rpool.tile([128, S], F, tag="sbc", name=f"sb{h}")
                        nc.gpsimd.partition_broadcast(sbc, srow)
                        rb = rpool.tile([128, S], F, tag="rb", name=f"rb{h}")
                        nc.vector.reciprocal_approx_fast(out=rb, in_=sbc)
                        rbh = rpool.tile([128, S], F16, tag="rbh", name=f"rh{h}")
                        nc.scalar.copy(out=rbh, in_=rb)
                        for kt in range(KT):
                            att_sb = apool.tile([128, S], F16, tag="att", name=f"at{h}_{kt}")
                            eng = nc.gpsimd if kt in (1, 3, 5, 7) else nc.vector
                            eng.tensor_tensor(
                                out=att_sb, in0=e_tiles[h][kt], in1=rbh, op=OP.mult)
                            nc.sync.dma_start(out=d["attT"][h, kt * 128:(kt + 1) * 128, :], in_=att_sb)
                        nc.vector.tensor_tensor(
                            out=oav[h], in0=av[h][0:DEP, :], in1=rb[0:DEP, :], op=OP.mult)
                        if _DEBUG:
                            av_dump = apool.tile([65, S], F, tag="avdump", name=f"avd{h}")
                            nc.scalar.activation(out=av_dump, in_=av[h], func=AF.Copy, scale=1.0)
                            nc.sync.dma_start(out=d["dbg_av"][h], in_=av_dump)
                            nc.sync.dma_start(out=d["dbg_oav"][h], in_=oav[h].bitcast(F))

        # ---------------- phase C: output projection ----------------
        with tc.tile_pool(name="osb", bufs=3) as opool, \
             tc.tile_pool(name="psO", bufs=4, space="PSUM") as psO:
            for qt in range(NT):
                out_sb = opool.tile([128, DIN], F, tag="osb")
                for nch in range(2):
                    po = psO.tile([128, 512], F, tag="po")
                    for h in range(NH):
                        nc.tensor.matmul(
                            po,
                            oav[h][:, qt * 128:(qt + 1) * 128],
                            wo_s[:, h, nch * 512:(nch + 1) * 512],
                            start=(h == 0), stop=(h == NH - 1))
                    nc.scalar.activation(
                        out=out_sb[:, nch * 512:(nch + 1) * 512], in_=po,
                        func=AF.Copy, scale=1.0)
                nc.sync.dma_start(out=d["outp"][qt * 128:(qt + 1) * 128, :], in_=out_sb)


def _get_nc():
    if "nc" not in _CACHE:
        _CACHE["nc"] = _build()
    return _CACHE["nc"]


def kernel(v, k, q, mask, Wq0, bq0, Wk0, bk0, Wv, bv, Wo, bo):
    v = np.asarray(v, dtype=np.float32)
    k = np.asarray(k, dtype=np.float32)
    q = np.asarray(q, dtype=np.float32)
    mask = np.asarray(mask)
    Wq0 = np.asarray(Wq0, dtype=np.float32)
    Wk0 = np.asarray(Wk0, dtype=np.float32)
    Wv = np.asarray(Wv, dtype=np.float32)
    Wo = np.asarray(Wo, dtype=np.float32)
    bq0 = np.asarray(bq0, dtype=np.float32)
    bk0 = np.asarray(bk0, dtype=np.float32)
    bv = np.asarray(bv, dtype=np.float32)
    bo = np.asarray(bo, dtype=np.float32)
    B = v.shape[0]
    HTOT = 16

    nc = _get_nc()

    per_batch = []
    for b in range(B):
        per_batch.append({
            "qT": np.ascontiguousarray(q[b, 1:, :].T),
            "kT": np.ascontiguousarray(k[b, :-1, :].T),
            "vT": np.ascontiguousarray(v[b].T).astype(np.float16),
            "keepT": np.ascontiguousarray((1 - mask[b]).T).astype(np.float16),
        })
    in_maps = []
    for c in range(8):
        b, g = c // 4, c % 4
        cols = slice(g * DCOL, (g + 1) * DCOL)
        m = dict(per_batch[b])
        m["wq"] = np.ascontiguousarray(Wq0[:, cols])
        m["wk"] = np.ascontiguousarray(Wk0[:, cols])
        m["wv"] = np.ascontiguousarray(Wv[:, cols]).astype(np.float16)
        m["wo"] = np.ascontiguousarray(Wo[cols, :])
        m["bq"] = np.ascontiguousarray(bq0[cols].reshape(NH, DEP).T)
        m["bk"] = np.ascontiguousarray(bk0[cols].reshape(2, 128).T)
        in_maps.append(m)

    res = run_bass_kernel_spmd(nc, in_maps, core_ids=list(range(8)))

    att = np.empty((B, HTOT, S, S), dtype=np.float64)
    out = np.empty((B, S, DIN), dtype=np.float64)
    bias_row = (bv.astype(np.float64) @ Wo.astype(np.float64)) + bo.astype(np.float64)
    for b in range(B):
        acc = None
        for g in range(4):
            r = res.results[b * 4 + g]
            attT = r["attT"]
            for hl in range(NH):
                att[b, g * NH + hl] = attT[hl].T
            acc = r["outp"].astype(np.float64) if acc is None else acc + r["outp"]
        out[b] = acc + bias_row[None, :]
    return out, att


# revision 19
# speedup vs baseline: 1.4019x; 1.0522x over previous
"""Trainium2 Bass kernel for nn_MultiHeadAttention_67757404062370.

Sharding: data-parallel over batch (2) x tensor-parallel over heads (4 groups
of 4 heads) = 8 NeuronCores. Core c handles batch c//4, heads 4*(c%4)..4*(c%4)+3.

Device-side per core (transposed layout throughout):
  kk^T = Wk_g^T k^T (+bk)  [128=2 heads x 64, 2, seq] fp32r
  qq^T = Wq_g^T q^T (+bq)  [128, 4, seq] fp32r, zero-padded in the other
         head's 64 rows so the scores contraction can use K=128 (K=64
         matmuls never warm the PE clock gate).
  vv   = v Wv_g  [seq, 4*65] fp16 with ones columns
  s^T[k,q] = kkT_tile.T @ qqT_pad   (K=128, fp32r)
  u = exp(s/8) fp16; t = max(u,1); e = t*keep  (exp(relu(x)) == max(exp(x),1))
  av^T (+ sums row via ones cols) = vv_aug.T @ e   (fp16, K=128)
  att^T = e * (1/sums) fp16 ; oav^T = av^T * (1/sums) fp32r
  outp[q,:] = sum_h oav_h^T.T @ Wo_rows_h  (K=64 per head, fp32r)
Host: gather, transpose att views, sum outp over the 4 head-group cores per
batch, add (bv @ Wo + bo), cast to float64.
"""

import sys

if "/opt/trn_rl_repo" not in sys.path:
    sys.path.insert(0, "/opt/trn_rl_repo")

import numpy as np

import concourse.bacc as bacc
import concourse.tile as tile
from concourse import mybir
from concourse.bass_utils import run_bass_kernel_spmd

F = mybir.dt.float32
FR = mybir.dt.float32r
F16 = mybir.dt.float16
AF = mybir.ActivationFunctionType
OP = mybir.AluOpType

S = 1024
DIN = 1024
NH = 4
DEP = 64
DCOL = NH * DEP
NT = S // 128
KT = DIN // 128

_CACHE = {}
_DEBUG = False


def _build():
    nc = bacc.Bacc("TRN2", target_bir_lowering=False, debug=False, num_devices=8)

    d = {}
    d["qT"] = nc.dram_tensor("qT", [DIN, S], FR, kind="ExternalInput").ap()
    d["kT"] = nc.dram_tensor("kT", [DIN, S], FR, kind="ExternalInput").ap()
    d["vT"] = nc.dram_tensor("vT", [DIN, S], F16, kind="ExternalInput").ap()
    d["keepT"] = nc.dram_tensor("keepT", [S, S], F16, kind="ExternalInput").ap()
    d["wq"] = nc.dram_tensor("wq", [DIN, DCOL], FR, kind="ExternalInput").ap()
    d["wk"] = nc.dram_tensor("wk", [DIN, DCOL], FR, kind="ExternalInput").ap()
    d["wv"] = nc.dram_tensor("wv", [DIN, DCOL], F16, kind="ExternalInput").ap()
    d["wo"] = nc.dram_tensor("wo", [DCOL, DIN], FR, kind="ExternalInput").ap()
    d["bq"] = nc.dram_tensor("bq", [DEP, NH], F, kind="ExternalInput").ap()
    d["bk"] = nc.dram_tensor("bk", [128, 2], F, kind="ExternalInput").ap()
    d["attT"] = nc.dram_tensor("attT", [NH, S, S], F16, kind="ExternalOutput").ap()
    d["outp"] = nc.dram_tensor("outp", [S, DIN], F, kind="ExternalOutput").ap()
    if _DEBUG:
        d["dbg_av"] = nc.dram_tensor("dbg_av", [NH, 65, S], F, kind="ExternalOutput").ap()
        d["dbg_oav"] = nc.dram_tensor("dbg_oav", [NH, DEP, S], F, kind="ExternalOutput").ap()

    with tile.TileContext(nc) as tc:
        _emit(nc, tc, d)
    nc.compile()
    return nc


def _emit(nc, tc, d):
    from contextlib import ExitStack

    ctx = ExitStack()
    with ctx:
        # ---------------- persistent tiles ----------------
        persist = ctx.enter_context(tc.tile_pool(name="persist", bufs=1))
        # q proj, zero-padded per head to a full 128-row contraction
        qqT = persist.tile([128, NH, S], FR, tag="qqT")
        # k proj, two heads stacked per dcol-tile
        kkT = persist.tile([128, 2, S], FR, tag="kkT")
        vv = persist.tile([128, NT, NH * 65], F16, tag="vv")
        wo_s = persist.tile([128, NH, DIN], FR, tag="wo")
        keep_s = [persist.tile([128, S], F16, tag=f"keep{i}", name=f"keep{i}") for i in range(KT)]
        bq_s = persist.tile([DEP, NH], F, tag="bq")
        bk_s = persist.tile([128, 2], F, tag="bk")
        ones_sb = persist.tile([128, NH, 1], F, tag="ones")
        nc.vector.memset(ones_sb, 1.0)
        oav = [persist.tile([128, S], FR, tag=f"oav{h}", name=f"oav{h}") for h in range(NH)]

        nc.sync.dma_start(out=bq_s, in_=d["bq"])
        nc.sync.dma_start(out=bk_s, in_=d["bk"])

        # ---------------- phase A: q/k projections ----------------
        with tc.tile_pool(name="wqk", bufs=1) as wpool, \
             tc.tile_pool(name="projin", bufs=4) as pin:
            wq_s = wpool.tile([128, KT, DCOL], FR, tag="wq")
            wk_s = wpool.tile([128, KT, DCOL], FR, tag="wk")
            with tc.tile_pool(name="psA1", bufs=4, space="PSUM") as psA1:
                ps_q = [psA1.tile([128, S], F, tag="psA1", name=f"psq{i}") for i in range(2)]
                ps_k = [psA1.tile([128, S], F, tag="psA1", name=f"psk{i}") for i in range(2)]
                for kt in range(KT):
                    rows = slice(kt * 128, (kt + 1) * 128)
                    nc.sync.dma_start(out=wq_s[:, kt, :], in_=d["wq"][rows, :])
                    nc.sync.dma_start(out=wk_s[:, kt, :], in_=d["wk"][rows, :])
                    qin = pin.tile([128, S], FR, tag="pin")
                    nc.sync.dma_start(out=qin, in_=d["qT"][rows, :])
                    kin = pin.tile([128, S], FR, tag="pin")
                    nc.sync.dma_start(out=kin, in_=d["kT"][rows, :])
                    for dc in range(2):
                        for cc in range(2):
                            sl = slice(cc * 512, (cc + 1) * 512)
                            nc.tensor.matmul(
                                ps_q[dc][:, sl],
                                wq_s[:, kt, dc * 128:(dc + 1) * 128],
                                qin[:, sl],
                                start=(kt == 0), stop=(kt == KT - 1))
                            nc.tensor.matmul(
                                ps_k[dc][:, sl],
                                wk_s[:, kt, dc * 128:(dc + 1) * 128],
                                kin[:, sl],
                                start=(kt == 0), stop=(kt == KT - 1))
                # remaining input loads queue behind the q/k stream
                for kt in range(KT):
                    nc.sync.dma_start(
                        out=keep_s[kt],
                        in_=d["keepT"][kt * 128:(kt + 1) * 128, :])
                # evacuate: kk full tiles w/ bias; qq per head w/ zero padding
                for dc in range(2):
                    nc.scalar.activation(
                        out=kkT[:, dc, :], in_=ps_k[dc],
                        func=AF.Identity, bias=bk_s[:, dc:dc + 1], scale=1.0)
                    for hf in range(2):
                        h = dc * 2 + hf
                        rw = slice(hf * DEP, (hf + 1) * DEP)
                        zw = slice((1 - hf) * DEP, (2 - hf) * DEP)
                        nc.scalar.activation(
                            out=qqT[rw, h, :], in_=ps_q[dc][rw, :],
                            func=AF.Identity, bias=bq_s[:, h:h + 1], scale=1.0)
                        # zero rows: Identity(0*x + 0)
                        nc.scalar.activation(
                            out=qqT[zw, h, :], in_=ps_q[dc][zw, :],
                            func=AF.Identity, bias=0.0, scale=0.0)
                for h in range(NH):
                    nc.scalar.activation(
                        out=oav[h][DEP:128, :], in_=ps_q[0][DEP:128, :],
                        func=AF.Identity, bias=0.0, scale=0.0)

        # ---------------- v input stream (fp16) ----------------
        with tc.tile_pool(name="wvp", bufs=1) as wvpool, \
             tc.tile_pool(name="vtp", bufs=1) as vtp:
            wv_s = wvpool.tile([128, KT, DCOL], F16, tag="wv")
            vt_s = [vtp.tile([128, S], F16, tag=f"vt{i}", name=f"vt{i}") for i in range(KT)]
            for kt in range(KT):
                rows = slice(kt * 128, (kt + 1) * 128)
                nc.sync.dma_start(out=wv_s[:, kt, :], in_=d["wv"][rows, :])
                nc.sync.dma_start(out=vt_s[kt], in_=d["vT"][rows, :])
            for h in range(NH):
                # duplicate each head's 64 Wo rows into both halves so the
                # phase-C contraction can be K=128 (zeros in oav rows 64:128
                # kill the duplicate; K=64 matmuls never warm the PE clock)
                nc.sync.dma_start(
                    out=wo_s[0:DEP, h, :], in_=d["wo"][h * DEP:(h + 1) * DEP, :])
                nc.sync.dma_start(
                    out=wo_s[DEP:128, h, :], in_=d["wo"][h * DEP:(h + 1) * DEP, :])

            # ---------------- phase B: heads in pairs ----------------
            with tc.tile_pool(name="escore", bufs=20) as epool, \
                 tc.tile_pool(name="utile", bufs=3) as upool, \
                 tc.tile_pool(name="attsb", bufs=5) as apool, \
                 tc.tile_pool(name="rtiles", bufs=2) as rpool, \
                 tc.tile_pool(name="psS", bufs=2, space="PSUM") as psS, \
                 tc.tile_pool(name="psAV", bufs=2, space="PSUM") as psAV:
                for hp in range(NH // 2):
                    heads = (2 * hp, 2 * hp + 1)
                    e_tiles = {h: [] for h in heads}
                    # scores + exp + mask for both heads of the pair
                    for kt in range(KT):
                        for h in heads:
                            ps = psS.tile([128, S], F, tag="score", name=f"sc{h}_{kt}")
                            for cc in range(2):
                                sl = slice(cc * 512, (cc + 1) * 512)
                                nc.tensor.matmul(
                                    ps[:, sl],
                                    kkT[:, hp, kt * 128:(kt + 1) * 128],
                                    qqT[:, h, sl],
                                    start=True, stop=True)
                            u = upool.tile([128, S], F16, tag="u", name=f"u{h}_{kt}")
                            nc.scalar.activation(out=u, in_=ps, func=AF.Exp, scale=0.125)
                            t = upool.tile([128, S], F16, tag="t", name=f"t{h}_{kt}")
                            nc.vector.tensor_scalar_max(out=t, in0=u, scalar1=1.0)
                            e = epool.tile([128, S], F16, tag="e", name=f"e{h}_{kt}")
                            eng = nc.gpsimd if kt in (2, 5) else nc.vector
                            eng.tensor_tensor(
                                out=e, in0=t, in1=keep_s[kt], op=OP.mult)
                            e_tiles[h].append(e)
                    if hp == 0:
                        # vv projection: sequential per seq-tile, slots shared
                        # with the scores tag (PE reaches here after S0)
                        for st in range(NT):
                            pv = psS.tile([128, 256], F, tag="score", name=f"psv{st}")
                            for kt in range(KT):
                                nc.tensor.matmul(
                                    pv,
                                    vt_s[kt][:, st * 128:(st + 1) * 128],
                                    wv_s[:, kt, :],
                                    start=(kt == 0), stop=(kt == KT - 1))
                            dst = vv[:, st, :].rearrange("p (h x) -> p h x", h=NH)
                            nc.scalar.activation(
                                out=dst[:, :, 0:DEP],
                                in_=pv.rearrange("p (h x) -> p h x", h=NH),
                                func=AF.Copy, scale=1.0)
                            nc.vector.tensor_copy(out=dst[:, :, DEP:DEP + 1], in_=ones_sb)
                    # AV matmuls per head
                    av = {}
                    for h in heads:
                        av[h] = psAV.tile([65, S], F, tag="av", name=f"av{h}")
                        for kt in range(KT):
                            for cc in range(2):
                                sl = slice(cc * 512, (cc + 1) * 512)
                                nc.tensor.matmul(
                                    av[h][:, sl],
                                    vv[:, kt, h * 65:(h + 1) * 65],
                                    e_tiles[h][kt][:, sl],
                                    start=(kt == 0), stop=(kt == KT - 1))
                    for h in heads:
                        srow = rpool.tile([1, S], F, tag="srow", name=f"sr{h}")
                        nc.scalar.activation(out=srow, in_=av[h][64:65, :], func=AF.Copy, scale=1.0)
                        sbc = rpool.tile([128, S], F, tag="sbc", name=f"sb{h}")
                        nc.gpsimd.partition_broadcast(sbc, srow)
                        rb = rpool.tile([128, S], F, tag="rb", name=f"rb{h}")
                        nc.vector.reciprocal_approx_fast(out=rb, in_=sbc)
                        rbh = rpool.tile([128, S], F16, tag="rbh", name=f"rh{h}")
                        nc.vector.tensor_copy(out=rbh, in_=rb)
                        for kt in range(KT):
                            att_sb = apool.tile([128, S], F16, tag="att", name=f"at{h}_{kt}")
                            eng = nc.gpsimd if kt in (3, 7) else nc.vector
                            eng.tensor_tensor(
                                out=att_sb, in0=e_tiles[h][kt], in1=rbh, op=OP.mult)
                            nc.sync.dma_start(out=d["attT"][h, kt * 128:(kt + 1) * 128, :], in_=att_sb)
                        nc.vector.tensor_tensor(
                            out=oav[h][0:DEP, :], in0=av[h][0:DEP, :], in1=rb[0:DEP, :], op=OP.mult)
                        if _DEBUG:
                            av_dump = apool.tile([65, S], F, tag="avdump", name=f"avd{h}")
                            nc.scalar.activation(out=av_dump, in_=av[h], func=AF.Copy, scale=1.0)
                            nc.sync.dma_start(out=d["dbg_av"][h], in_=av_dump)
                            nc.sync.dma_start(out=d["dbg_oav"][h], in_=oav[h][0:DEP, :].bitcast(F))

        # ---------------- phase C: output projection ----------------
        with tc.tile_pool(name="osb", bufs=3) as opool, \
             tc.tile_pool(name="psO", bufs=4, space="PSUM") as psO:
            for qt in range(NT):
                out_sb = opool.tile([128, DIN], F, tag="osb")
                for nch in range(2):
                    po = psO.tile([128, 512], F, tag="po")
                    for h in range(NH):
                        nc.tensor.matmul(
                            po,
                            oav[h][:, qt * 128:(qt + 1) * 128],
                            wo_s[:, h, nch * 512:(nch + 1) * 512],
                            start=(h == 0), stop=(h == NH - 1))
                    nc.scalar.activation(
                        out=out_sb[:, nch * 512:(nch + 1) * 512], in_=po,
                        func=AF.Copy, scale=1.0)
                nc.sync.dma_start(out=d["outp"][qt * 128:(qt + 1) * 128, :], in_=out_sb)


def _get_nc():
    if "nc" not in _CACHE:
        _CACHE["nc"] = _build()
    return _CACHE["nc"]


def kernel(v, k, q, mask, Wq0, bq0, Wk0, bk0, Wv, bv, Wo, bo):
    v = np.asarray(v, dtype=np.float32)
    k = np.asarray(k, dtype=np.float32)
    q = np.asarray(q, dtype=np.float32)
    mask = np.asarray(mask)
    Wq0 = np.asarray(Wq0, dtype=np.float32)
    Wk0 = np.asarray(Wk0, dtype=np.float32)
    Wv = np.asarray(Wv, dtype=np.float32)
    Wo = np.asarray(Wo, dtype=np.float32)
    bq0 = np.asarray(bq0, dtype=np.float32)
    bk0 = np.asarray(bk0, dtype=np.float32)
    bv = np.asarray(bv, dtype=np.float32)
    bo = np.asarray(bo, dtype=np.float32)
    B = v.shape[0]
    HTOT = 16

    nc = _get_nc()

    per_batch = []
    for b in range(B):
        per_batch.append({
            "qT": np.ascontiguousarray(q[b, 1:, :].T),
            "kT": np.ascontiguousarray(k[b, :-1, :].T),
            "vT": np.ascontiguousarray(v[b].T).astype(np.float16),
            "keepT": np.ascontiguousarray((1 - mask[b]).T).astype(np.float16),
        })
    in_maps = []
    for c in range(8):
        b, g = c // 4, c % 4
        cols = slice(g * DCOL, (g + 1) * DCOL)
        m = dict(per_batch[b])
        m["wq"] = np.ascontiguousarray(Wq0[:, cols])
        m["wk"] = np.ascontiguousarray(Wk0[:, cols])
        m["wv"] = np.ascontiguousarray(Wv[:, cols]).astype(np.float16)
        m["wo"] = np.ascontiguousarray(Wo[cols, :])
        m["bq"] = np.ascontiguousarray(bq0[cols].reshape(NH, DEP).T)
        m["bk"] = np.ascontiguousarray(bk0[cols].reshape(2, 128).T)
        in_maps.append(m)

    res = run_bass_kernel_spmd(nc, in_maps, core_ids=list(range(8)))

    att = np.empty((B, HTOT, S, S), dtype=np.float64)
    out = np.empty((B, S, DIN), dtype=np.float64)
    bias_row = (bv.astype(np.float64) @ Wo.astype(np.float64)) + bo.astype(np.float64)
    for b in range(B):
        acc = None
        for g in range(4):
            r = res.results[b * 4 + g]
            attT = r["attT"]
            for hl in range(NH):
                att[b, g * NH + hl] = attT[hl].T
            acc = r["outp"].astype(np.float64) if acc is None else acc + r["outp"]
        out[b] = acc + bias_row[None, :]
    return out, att


# revision 21
# speedup vs baseline: 1.5915x; 1.1352x over previous
"""Trainium2 Bass kernel for nn_MultiHeadAttention_67757404062370.

Sharding: data-parallel over batch (2) x tensor-parallel over heads (4 groups
of 4 heads) = 8 NeuronCores. Core c handles batch c//4, heads 4*(c%4)..4*(c%4)+3.

Device-side per core (transposed layout throughout):
  kk^T = Wk_g^T k^T (+bk)  [128=2 heads x 64, 2, seq] fp32r
  qq^T = Wq_g^T q^T (+bq)  [128, 4, seq] fp32r, zero-padded in the other
         head's 64 rows so the scores contraction can use K=128 (K=64
         matmuls never warm the PE clock gate).
  vv   = v Wv_g  [seq, 4*65] fp16 with ones columns
  s^T[k,q] = kkT_tile.T @ qqT_pad   (K=128, fp32r)
  u = exp(s/8) fp16; t = max(u,1); e = t*keep  (exp(relu(x)) == max(exp(x),1))
  av^T (+ sums row via ones cols) = vv_aug.T @ e   (fp16, K=128)
  att^T = e * (1/sums) fp16 ; oav^T = av^T * (1/sums) fp32r
  outp[q,:] = sum_h oav_h^T.T @ Wo_rows_h  (K=64 per head, fp32r)
Host: gather, transpose att views, sum outp over the 4 head-group cores per
batch, add (bv @ Wo + bo), cast to float64.
"""

import sys

if "/opt/trn_rl_repo" not in sys.path:
    sys.path.insert(0, "/opt/trn_rl_repo")

import numpy as np

import concourse.bacc as bacc
import concourse.tile as tile
from concourse import mybir
from concourse.bass_utils import run_bass_kernel_spmd

F = mybir.dt.float32
FR = mybir.dt.float32r
F16 = mybir.dt.float16
AF = mybir.ActivationFunctionType
OP = mybir.AluOpType

S = 1024
DIN = 1024
NH = 4
DEP = 64
DCOL = NH * DEP
NT = S // 128
KT = DIN // 128

_CACHE = {}
_DEBUG = False


def _build():
    nc = bacc.Bacc("TRN2", target_bir_lowering=False, debug=False, num_devices=8)

    d = {}
    d["qT"] = nc.dram_tensor("qT", [DIN, S], FR, kind="ExternalInput").ap()
    d["kT"] = nc.dram_tensor("kT", [DIN, S], FR, kind="ExternalInput").ap()
    d["vT"] = nc.dram_tensor("vT", [DIN, S], F16, kind="ExternalInput").ap()
    d["keepT"] = nc.dram_tensor("keepT", [S, S], F16, kind="ExternalInput").ap()
    d["wq"] = nc.dram_tensor("wq", [DIN, DCOL], FR, kind="ExternalInput").ap()
    d["wk"] = nc.dram_tensor("wk", [DIN, DCOL], FR, kind="ExternalInput").ap()
    d["wv"] = nc.dram_tensor("wv", [DIN, DCOL], F16, kind="ExternalInput").ap()
    d["wo"] = nc.dram_tensor("wo", [DCOL, DIN], FR, kind="ExternalInput").ap()
    d["bq"] = nc.dram_tensor("bq", [DEP, NH], F, kind="ExternalInput").ap()
    d["bk"] = nc.dram_tensor("bk", [128, 2], F, kind="ExternalInput").ap()
    d["attT"] = nc.dram_tensor("attT", [NH, S, S], F16, kind="ExternalOutput").ap()
    d["outp"] = nc.dram_tensor("outp", [S, DIN], F, kind="ExternalOutput").ap()
    if _DEBUG:
        d["dbg_av"] = nc.dram_tensor("dbg_av", [NH, 65, S], F, kind="ExternalOutput").ap()
        d["dbg_oav"] = nc.dram_tensor("dbg_oav", [NH, DEP, S], F, kind="ExternalOutput").ap()

    with tile.TileContext(nc) as tc:
        _emit(nc, tc, d)
    nc.compile()
    return nc


def _emit(nc, tc, d):
    from contextlib import ExitStack

    ctx = ExitStack()
    with ctx:
        # ---------------- persistent tiles ----------------
        persist = ctx.enter_context(tc.tile_pool(name="persist", bufs=1))
        # q proj, zero-padded per head to a full 128-row contraction
        qqT = persist.tile([128, NH, S], FR, tag="qqT")
        # k proj, two heads stacked per dcol-tile
        kkT = persist.tile([128, 2, S], FR, tag="kkT")
        vv = persist.tile([128, NT, NH * 65], F16, tag="vv")
        wo_s = persist.tile([128, NH, DIN], FR, tag="wo")
        keep_s = [persist.tile([128, S], F16, tag=f"keep{i}", name=f"keep{i}") for i in range(KT)]
        bq_s = persist.tile([DEP, NH], F, tag="bq")
        bk_s = persist.tile([128, 2], F, tag="bk")
        ones_sb = persist.tile([128, NH, 1], F, tag="ones")
        nc.vector.memset(ones_sb, 1.0)
        oav = [persist.tile([128, S], FR, tag=f"oav{h}", name=f"oav{h}") for h in range(NH)]

        nc.sync.dma_start(out=bq_s, in_=d["bq"])
        nc.sync.dma_start(out=bk_s, in_=d["bk"])

        # ---------------- phase A: q/k projections ----------------
        with tc.tile_pool(name="wqk", bufs=1) as wpool, \
             tc.tile_pool(name="projin", bufs=4) as pin:
            wq_s = wpool.tile([128, KT, DCOL], FR, tag="wq")
            wk_s = wpool.tile([128, KT, DCOL], FR, tag="wk")
            with tc.tile_pool(name="psA1", bufs=4, space="PSUM") as psA1:
                ps_q = [psA1.tile([128, S], F, tag="psA1", name=f"psq{i}") for i in range(2)]
                ps_k = [psA1.tile([128, S], F, tag="psA1", name=f"psk{i}") for i in range(2)]
                nc.sync.dma_start(out=wq_s, in_=d["wq"].rearrange("(t p) c -> p t c", p=128))
                nc.sync.dma_start(out=wk_s, in_=d["wk"].rearrange("(t p) c -> p t c", p=128))
                for kt in range(KT):
                    rows = slice(kt * 128, (kt + 1) * 128)
                    qin = pin.tile([128, S], FR, tag="pin")
                    nc.sync.dma_start(out=qin, in_=d["qT"][rows, :])
                    kin = pin.tile([128, S], FR, tag="pin")
                    nc.sync.dma_start(out=kin, in_=d["kT"][rows, :])
                    for dc in range(2):
                        for cc in range(2):
                            sl = slice(cc * 512, (cc + 1) * 512)
                            nc.tensor.matmul(
                                ps_q[dc][:, sl],
                                wq_s[:, kt, dc * 128:(dc + 1) * 128],
                                qin[:, sl],
                                start=(kt == 0), stop=(kt == KT - 1))
                            nc.tensor.matmul(
                                ps_k[dc][:, sl],
                                wk_s[:, kt, dc * 128:(dc + 1) * 128],
                                kin[:, sl],
                                start=(kt == 0), stop=(kt == KT - 1))
                # remaining input loads queue behind the q/k stream
                for kt in range(KT):
                    nc.sync.dma_start(
                        out=keep_s[kt],
                        in_=d["keepT"][kt * 128:(kt + 1) * 128, :])
                # evacuate: kk full tiles w/ bias; qq per head w/ zero padding
                for dc in range(2):
                    nc.scalar.activation(
                        out=kkT[:, dc, :], in_=ps_k[dc],
                        func=AF.Identity, bias=bk_s[:, dc:dc + 1], scale=1.0)
                    for hf in range(2):
                        h = dc * 2 + hf
                        rw = slice(hf * DEP, (hf + 1) * DEP)
                        zw = slice((1 - hf) * DEP, (2 - hf) * DEP)
                        nc.scalar.activation(
                            out=qqT[rw, h, :], in_=ps_q[dc][rw, :],
                            func=AF.Identity, bias=bq_s[:, h:h + 1], scale=1.0)
                        # zero rows via DVE (idle during phase A): 0*x
                        nc.vector.tensor_scalar_mul(
                            out=qqT[zw, h, :], in0=ps_q[dc][zw, :], scalar1=0.0)

        # ---------------- v input stream (fp16) ----------------
        with tc.tile_pool(name="wvp", bufs=1) as wvpool, \
             tc.tile_pool(name="vtp", bufs=1) as vtp:
            wv_s = wvpool.tile([128, KT, DCOL], F16, tag="wv")
            vt_s = [vtp.tile([128, S], F16, tag=f"vt{i}", name=f"vt{i}") for i in range(KT)]
            nc.sync.dma_start(out=wv_s, in_=d["wv"].rearrange("(t p) c -> p t c", p=128))
            for kt in range(KT):
                rows = slice(kt * 128, (kt + 1) * 128)
                nc.sync.dma_start(out=vt_s[kt], in_=d["vT"][rows, :])
            for h in range(NH):
                # duplicate each head's 64 Wo rows into both halves so the
                # phase-C contraction can be K=128 (zeros in oav rows 64:128
                # kill the duplicate; K=64 matmuls never warm the PE clock)
                nc.sync.dma_start(
                    out=wo_s[0:DEP, h, :], in_=d["wo"][h * DEP:(h + 1) * DEP, :])
                nc.sync.dma_start(
                    out=wo_s[DEP:128, h, :], in_=d["wo"][h * DEP:(h + 1) * DEP, :])

            # ---------------- phase B: heads in pairs ----------------
            with tc.tile_pool(name="escore", bufs=20) as epool, \
                 tc.tile_pool(name="utile", bufs=3) as upool, \
                 tc.tile_pool(name="attsb", bufs=5) as apool, \
                 tc.tile_pool(name="rtiles", bufs=2) as rpool, \
                 tc.tile_pool(name="psS", bufs=2, space="PSUM") as psS, \
                 tc.tile_pool(name="psAV", bufs=2, space="PSUM") as psAV:
                for hp in range(NH // 2):
                    heads = (2 * hp, 2 * hp + 1)
                    e_tiles = {h: [] for h in heads}
                    # scores + exp + mask for both heads of the pair
                    for kt in range(KT):
                        for h in heads:
                            ps = psS.tile([128, S], F, tag="score", name=f"sc{h}_{kt}")
                            for cc in range(2):
                                sl = slice(cc * 512, (cc + 1) * 512)
                                nc.tensor.matmul(
                                    ps[:, sl],
                                    kkT[:, hp, kt * 128:(kt + 1) * 128],
                                    qqT[:, h, sl],
                                    start=True, stop=True)
                            u = upool.tile([128, S], F16, tag="u", name=f"u{h}_{kt}")
                            nc.scalar.activation(out=u, in_=ps, func=AF.Exp, scale=0.125)
                            t = upool.tile([128, S], F16, tag="t", name=f"t{h}_{kt}")
                            nc.vector.tensor_scalar_max(out=t, in0=u, scalar1=1.0)
                            e = epool.tile([128, S], F16, tag="e", name=f"e{h}_{kt}")
                            nc.vector.tensor_tensor(
                                out=e, in0=t, in1=keep_s[kt], op=OP.mult)
                            e_tiles[h].append(e)
                    if hp == 0:
                        # vv projection: sequential per seq-tile, slots shared
                        # with the scores tag (PE reaches here after S0)
                        for st in range(NT):
                            pv = psS.tile([128, 256], F, tag="score", name=f"psv{st}")
                            for kt in range(KT):
                                nc.tensor.matmul(
                                    pv,
                                    vt_s[kt][:, st * 128:(st + 1) * 128],
                                    wv_s[:, kt, :],
                                    start=(kt == 0), stop=(kt == KT - 1))
                            dst = vv[:, st, :].rearrange("p (h x) -> p h x", h=NH)
                            nc.scalar.activation(
                                out=dst[:, :, 0:DEP],
                                in_=pv.rearrange("p (h x) -> p h x", h=NH),
                                func=AF.Copy, scale=1.0)
                            nc.vector.tensor_copy(out=dst[:, :, DEP:DEP + 1], in_=ones_sb)
                    # AV matmuls per head
                    av = {}
                    for h in heads:
                        av[h] = psAV.tile([65, S], F, tag="av", name=f"av{h}")
                        for kt in range(KT):
                            for cc in range(2):
                                sl = slice(cc * 512, (cc + 1) * 512)
                                nc.tensor.matmul(
                                    av[h][:, sl],
                                    vv[:, kt, h * 65:(h + 1) * 65],
                                    e_tiles[h][kt][:, sl],
                                    start=(kt == 0), stop=(kt == KT - 1))
                    for h in heads:
                        srow = rpool.tile([1, S], F, tag="srow", name=f"sr{h}")
                        nc.scalar.activation(out=srow, in_=av[h][64:65, :], func=AF.Copy, scale=1.0)
                        sbc = rpool.tile([128, S], F, tag="sbc", name=f"sb{h}")
                        nc.gpsimd.partition_broadcast(sbc, srow)
                        rb = rpool.tile([128, S], F, tag="rb", name=f"rb{h}")
                        nc.vector.reciprocal_approx_fast(out=rb, in_=sbc)
                        rbh = rpool.tile([128, S], F16, tag="rbh", name=f"rh{h}")
                        nc.vector.tensor_copy(out=rbh, in_=rb)
                        for kt in range(KT):
                            att_sb = apool.tile([128, S], F16, tag="att", name=f"at{h}_{kt}")
                            nc.vector.tensor_tensor(
                                out=att_sb, in0=e_tiles[h][kt], in1=rbh, op=OP.mult)
                            nc.sync.dma_start(out=d["attT"][h, kt * 128:(kt + 1) * 128, :], in_=att_sb)
                        nc.vector.tensor_tensor(
                            out=oav[h][0:DEP, :], in0=av[h][0:DEP, :], in1=rb[0:DEP, :], op=OP.mult)
                        if _DEBUG:
                            av_dump = apool.tile([65, S], F, tag="avdump", name=f"avd{h}")
                            nc.scalar.activation(out=av_dump, in_=av[h], func=AF.Copy, scale=1.0)
                            nc.sync.dma_start(out=d["dbg_av"][h], in_=av_dump)
                            nc.sync.dma_start(out=d["dbg_oav"][h], in_=oav[h][0:DEP, :].bitcast(F))

        # oav bottom halves zeroed (killed the duplicated Wo rows in phase C)
        for h in range(NH):
            nc.scalar.activation(
                out=oav[h][DEP:128, :], in_=kkT[DEP:128, 0, :],
                func=AF.Identity, bias=0.0, scale=0.0)

        # ---------------- phase C: output projection ----------------
        with tc.tile_pool(name="osb", bufs=3) as opool, \
             tc.tile_pool(name="psO", bufs=4, space="PSUM") as psO:
            for qt in range(NT):
                out_sb = opool.tile([128, DIN], F, tag="osb")
                for nch in range(2):
                    po = psO.tile([128, 512], F, tag="po")
                    for h in range(NH):
                        nc.tensor.matmul(
                            po,
                            oav[h][:, qt * 128:(qt + 1) * 128],
                            wo_s[:, h, nch * 512:(nch + 1) * 512],
                            start=(h == 0), stop=(h == NH - 1))
                    nc.scalar.activation(
                        out=out_sb[:, nch * 512:(nch + 1) * 512], in_=po,
                        func=AF.Copy, scale=1.0)
                nc.sync.dma_start(out=d["outp"][qt * 128:(qt + 1) * 128, :], in_=out_sb)


def _get_nc():
    if "nc" not in _CACHE:
        _CACHE["nc"] = _build()
    return _CACHE["nc"]


def kernel(v, k, q, mask, Wq0, bq0, Wk0, bk0, Wv, bv, Wo, bo):
    v = np.asarray(v, dtype=np.float32)
    k = np.asarray(k, dtype=np.float32)
    q = np.asarray(q, dtype=np.float32)
    mask = np.asarray(mask)
    Wq0 = np.asarray(Wq0, dtype=np.float32)
    Wk0 = np.asarray(Wk0, dtype=np.float32)
    Wv = np.asarray(Wv, dtype=np.float32)
    Wo = np.asarray(Wo, dtype=np.float32)
    bq0 = np.asarray(bq0, dtype=np.float32)
    bk0 = np.asarray(bk0, dtype=np.float32)
    bv = np.asarray(bv, dtype=np.float32)
    bo = np.asarray(bo, dtype=np.float32)
    B = v.shape[0]
    HTOT = 16

    nc = _get_nc()

    per_batch = []
    for b in range(B):
        per_batch.append({
            "qT": np.ascontiguousarray(q[b, 1:, :].T),
            "kT": np.ascontiguousarray(k[b, :-1, :].T),
            "vT": np.ascontiguousarray(v[b].T).astype(np.float16),
            "keepT": np.ascontiguousarray((1 - mask[b]).T).astype(np.float16),
        })
    in_maps = []
    for c in range(8):
        b, g = c // 4, c % 4
        cols = slice(g * DCOL, (g + 1) * DCOL)
        m = dict(per_batch[b])
        m["wq"] = np.ascontiguousarray(Wq0[:, cols])
        m["wk"] = np.ascontiguousarray(Wk0[:, cols])
        m["wv"] = np.ascontiguousarray(Wv[:, cols]).astype(np.float16)
        m["wo"] = np.ascontiguousarray(Wo[cols, :])
        m["bq"] = np.ascontiguousarray(bq0[cols].reshape(NH, DEP).T)
        m["bk"] = np.ascontiguousarray(bk0[cols].reshape(2, 128).T)
        in_maps.append(m)

    res = run_bass_kernel_spmd(nc, in_maps, core_ids=list(range(8)))

    att = np.empty((B, HTOT, S, S), dtype=np.float64)
    out = np.empty((B, S, DIN), dtype=np.float64)
    bias_row = (bv.astype(np.float64) @ Wo.astype(np.float64)) + bo.astype(np.float64)
    for b in range(B):
        acc = None
        for g in range(4):
            r = res.results[b * 4 + g]
            attT = r["attT"]
            for hl in range(NH):
                att[b, g * NH + hl] = attT[hl].T
            acc = r["outp"].astype(np.float64) if acc is None else acc + r["outp"]
        out[b] = acc + bias_row[None, :]
    return out, att
